# revision 1
# baseline (speedup 1.0000x reference)
"""Trainium2 Bass kernel for a dense transformer block (LN1 -> MHA -> LN2 -> MLP).

Sharding: 8 cores = (batch b in 0..3) x (sequence half in 0..1). Each core
computes the block output for its 1024 query tokens; K/V are computed for the
batch's full 2048 tokens on each core (replicated within the pair), so there
is zero cross-core communication.

Layout: on-chip activations are transposed ([feature, token]) so matmul
chains compose without transposes; the host transposes x per core and
transposes the per-core outputs back.

Dtypes: attention path bf16 (Q/K/V/probs), residuals fp32, MLP float32r
(full PE speed at N>=256, ~1e-4 matmul accuracy), LN stats via bf16 PE
ones-matmuls (rounding noise averages out across 1024 terms).
"""

import sys

if '/opt/trn_rl_repo' not in sys.path:
    sys.path.insert(0, '/opt/trn_rl_repo')

import numpy as np
import ml_dtypes

import concourse.tile as tile
import concourse.mybir as mybir
from concourse import bacc
from concourse.bass import ts
from concourse.bass_utils import run_bass_kernel_spmd

P = 128
F32 = mybir.dt.float32
F32R = mybir.dt.float32r
BF16 = mybir.dt.bfloat16
AF = mybir.ActivationFunctionType
EPS = 1e-6

B, S, D, H, MLP = 4, 2048, 1024, 16, 4096
N_CORES = 8


def _layernorm(nc, ones_h, ones_r, eps_t, p_bc, p_tmp, p_st, ps_st, src_fn, n_dc, Tn, TBn,
               g_t, b_t, out_fn, dram_src=None):
    """LayerNorm along the feature (partition-chunk) direction.

    src_fn(dc) -> [P, Tn] fp32 AP of a resident tile, or None with dram_src
    set to a [Dm, Tn] fp32 dram AP to stream chunks (two passes over dram).
    out_fn(dc) -> [P, Tn] dest AP (any dtype).
    Feature sums via PE ones-matmuls on bf16 casts.
    """
    n_tb = Tn // TBn
    inv_d = 1.0 / (n_dc * P)
    for tb in range(n_tb):
        sl = ts(tb, TBn)
        ps_m = ps_st.tile([1, TBn], F32, tag="ps_stat")
        ps_s = ps_st.tile([1, TBn], F32, tag="ps_stat")
        for dc in range(n_dc):
            st, sp = (dc == 0), (dc == n_dc - 1)
            if dram_src is not None:
                # f32r-typed chunk: serves the mean matmul directly (no cast)
                xc = p_tmp.tile([P, TBn], F32R, tag="ln_xc")
                nc.sync.dma_start(xc[:],
                                  dram_src[ts(dc, P), sl].bitcast(F32R))
                nc.tensor.matmul(ps_m[:], ones_r[:], xc[:], start=st, stop=sp)
                src_sl = xc[:].bitcast(F32)
            else:
                src_sl = src_fn(dc)[:, sl]
                xb = p_tmp.tile([P, TBn], BF16, tag="ln_xb")
                nc.vector.tensor_copy(xb[:], src_sl)
                nc.tensor.matmul(ps_m[:], ones_h[:], xb[:], start=st, stop=sp)
            xsq = p_tmp.tile([P, TBn], BF16, tag="ln_xsq")
            nc.scalar.activation(xsq[:], src_sl, AF.Square)
            nc.tensor.matmul(ps_s[:], ones_h[:], xsq[:], start=st, stop=sp)
        mean = p_st.tile([1, TBn], F32)
        nc.vector.tensor_scalar_mul(mean[:], ps_m[:], inv_d)
        ex2 = p_st.tile([1, TBn], F32)
        nc.vector.tensor_scalar_mul(ex2[:], ps_s[:], inv_d)
        var = p_st.tile([1, TBn], F32)
        nc.vector.tensor_mul(var[:], mean[:], mean[:])
        nc.vector.tensor_sub(var[:], ex2[:], var[:])
        std = p_st.tile([1, TBn], F32)
        nc.scalar.activation(std[:], var[:], AF.Sqrt, bias=eps_t[:, :])
        rstd = p_st.tile([1, TBn], F32)
        nc.vector.reciprocal(rstd[:], std[:])
        # chunked apply: broadcast per token-block so later consumers of this
        # token-block unblock as soon as it's written
        mean_bc = p_tmp.tile([P, TBn], F32, tag="ln_meanbc_c")
        rstd_bc = p_tmp.tile([P, TBn], F32, tag="ln_rstdbc_c")
        nc.gpsimd.partition_broadcast(mean_bc[:], mean[:])
        nc.gpsimd.partition_broadcast(rstd_bc[:], rstd[:])
        for dc in range(n_dc):
            t0 = p_tmp.tile([P, TBn], F32, tag="ln_xa")
            if dram_src is not None:
                nc.sync.dma_start(t0[:], dram_src[ts(dc, P), sl])
                nc.vector.tensor_sub(t0[:], t0[:], mean_bc[:])
            else:
                nc.vector.tensor_sub(t0[:], src_fn(dc)[:, sl], mean_bc[:])
            nc.vector.tensor_mul(t0[:], t0[:], rstd_bc[:])
            nc.scalar.activation(out_fn(dc)[:, sl], t0[:], AF.Identity,
                                 bias=b_t[:, dc:dc + 1],
                                 scale=g_t[:, dc:dc + 1])


def build_bass(T, Q, Dm, Hh, Mlp, n_cores, dbg=False):
    dh = Dm // Hh
    assert dh == 64, "head packing assumes DH=64"
    n_dc = Dm // P
    n_tk = T // P
    TB = min(512, T)
    n_tb = T // TB
    QB = min(512, Q)
    n_qb = Q // QB
    QQ = min(512, Q)
    n_qq = Q // QQ
    n_mo = Mlp // P
    n_hp = Hh // 2

    nc = bacc.Bacc("TRN2", target_bir_lowering=False, debug=False,
                   enable_asserts=False, num_devices=n_cores)

    def din(name, shape, dt):
        return nc.dram_tensor(name, shape, dt, kind="ExternalInput").ap()

    xT_d = din("xT", (Dm, T), F32)
    xqT_d = din("xqT", (Dm, Q), F32)
    g1_d, be1_d = din("g1", (Dm,), F32), din("be1", (Dm,), F32)
    g2_d, be2_d = din("g2", (Dm,), F32), din("be2", (Dm,), F32)
    wq_d, wk_d = din("wq16", (Dm, Dm), BF16), din("wk16", (Dm, Dm), BF16)
    wv_d, wo_d = din("wv16", (Dm, Dm), BF16), din("wo16", (Dm, Dm), BF16)
    w1_d = din("w1r", (Dm, Mlp), F32R)
    w2_d = din("w2r16", (Mlp, Dm), BF16)
    bq_d, bk_d = din("bq", (Dm,), F32), din("bk", (Dm,), F32)
    bv_d, bo_d = din("bv", (Dm,), F32), din("bo", (Dm,), F32)
    b1_d, b2_d = din("b1", (Mlp,), F32), din("b2", (Dm,), F32)
    ones_d = din("ones16", (P, 1), BF16)
    onesr_d = din("ones_r", (P, 1), F32R)
    yT_d = nc.dram_tensor("yT", (Dm, Q), F32, kind="ExternalOutput").ap()
    dbg_d = {}
    if dbg:
        for nm, shape, dt in [("dXN", (Dm, T), BF16), ("dXNQ", (Dm, Q), BF16),
                              ("dKT", (Dm, T), BF16), ("dQT", (Dm, Q), BF16),
                              ("dVT", (T, Dm), BF16), ("dCT", (Dm, Q), BF16),
                              ("dh2", (Dm, Q), F32), ("drbc", (P, Q), F32),
                              ("dexp", (T, Q), BF16)]:
            dbg_d[nm] = nc.dram_tensor(nm, shape, dt, kind="ExternalOutput").ap()

    with tile.TileContext(nc) as tc:
        with tc.tile_pool(name="const", bufs=1) as constp:
            ones_h = constp.tile([P, 1], BF16)
            nc.sync.dma_start(ones_h[:], ones_d[:, :])
            eps_t = constp.tile([1, 1], F32)
            nc.vector.memset(eps_t[:], EPS)
            ones_r = constp.tile([P, 1], F32R)
            nc.sync.dma_start(ones_r[:], onesr_d[:, :])
            ones_f = constp.tile([P, P], BF16)
            nc.vector.memset(ones_f[:], 1.0)

            def vec_tile(src, n, nm):
                t = constp.tile([P, n], F32, tag=nm, name=nm)
                nc.sync.dma_start(t[:], src.rearrange("(c p) -> p c", p=P))
                return t

            g1_t, be1_t = vec_tile(g1_d, n_dc, "g1"), vec_tile(be1_d, n_dc, "be1")
            g2_t, be2_t = vec_tile(g2_d, n_dc, "g2"), vec_tile(be2_d, n_dc, "be2")
            bq_t, bk_t = vec_tile(bq_d, n_dc, "bq"), vec_tile(bk_d, n_dc, "bk")
            bo_t, b2_t = vec_tile(bo_d, n_dc, "bo"), vec_tile(b2_d, n_dc, "b2")
            b1_t = vec_tile(b1_d, n_mo, "b1")
            # bv broadcast along free dim (V is [token, d_out])
            bv_row = constp.tile([1, Dm], F32)
            nc.sync.dma_start(bv_row[:, :], bv_d[None, :])
            bv_bc = constp.tile([P, Dm], F32)
            nc.gpsimd.partition_broadcast(bv_bc[:], bv_row[:])

            with tc.tile_pool(name="p_h2", bufs=1) as p_h2:
                XQ = p_h2.tile([P, n_dc, Q], F32)  # x_q, becomes h2

                with tc.tile_pool(name="p_kv", bufs=1) as p_kv:
                    KT = p_kv.tile([P, n_dc, T], BF16)
                    VT = p_kv.tile([P, n_tk, Dm], BF16)
                    QT = p_kv.tile([P, n_dc, Q], BF16)

                    # ---------- Phase 1: LN1 + QKV ----------
                    with tc.tile_pool(name="p_act", bufs=1) as p_act, \
                         tc.tile_pool(name="p_str", bufs=6) as p_str, \
                         tc.tile_pool(name="p_tmp", bufs=2) as p_tmp, \
                         tc.tile_pool(name="p_st", bufs=1) as p_st, \
                         tc.tile_pool(name="ps_st", bufs=2, space="PSUM") as ps_st, \
                         tc.tile_pool(name="ps_mm", bufs=6, space="PSUM") as ps_mm:

                        XN = p_act.tile([P, n_dc, T], BF16)
                        _layernorm(nc, ones_h, ones_r, eps_t, p_act, p_tmp, p_st, ps_st,
                                   None, n_dc, T, TB,
                                   g1_t, be1_t, lambda dc: XN[:, dc, :],
                                   dram_src=xT_d)
                        XNQ = p_act.tile([P, n_dc, Q], BF16)
                        _layernorm(nc, ones_h, ones_r, eps_t, p_act, p_tmp, p_st, ps_st,
                                   None, n_dc, Q, QB,
                                   g1_t, be1_t, lambda dc: XNQ[:, dc, :],
                                   dram_src=xqT_d)

                        if dbg:
                            for dc in range(n_dc):
                                nc.sync.dma_start(dbg_d["dXN"][ts(dc, P), :], XN[:, dc, :])
                                nc.sync.dma_start(dbg_d["dXNQ"][ts(dc, P), :], XNQ[:, dc, :])
                        # K^T: lhsT = Wk chunk, rhs = XN. Token-pair-block
                        # outer so K starts once LN1 finished the first half;
                        # each weight chunk feeds 2 matmuls.
                        ktg = 4
                        for tb0 in range(0, n_tb, ktg):
                            tbs = range(tb0, min(tb0 + ktg, n_tb))
                            for mo in range(n_dc):
                                pss = [ps_mm.tile([P, TB], F32, tag="ps_mm",
                                                  name="ps_mm") for _ in tbs]
                                for dc in range(n_dc):
                                    wt = p_str.tile([P, P], BF16, tag="wkq")
                                    nc.sync.dma_start(wt[:],
                                                      wk_d[ts(dc, P), ts(mo, P)])
                                    for i, tb in enumerate(tbs):
                                        nc.tensor.matmul(
                                            pss[i][:], wt[:], XN[:, dc, ts(tb, TB)],
                                            start=(dc == 0), stop=(dc == n_dc - 1))
                                for i, tb in enumerate(tbs):
                                    nc.vector.tensor_scalar_add(
                                        KT[:, mo, ts(tb, TB)], pss[i][:],
                                        bk_t[:, mo:mo + 1])
                        # Q^T from XNQ
                        for mo in range(n_dc):
                            pss = [ps_mm.tile([P, QB], F32, tag="ps_mm",
                                              name="ps_mm") for _ in range(n_qb)]
                            for dc in range(n_dc):
                                wt = p_str.tile([P, P], BF16, tag="wkq")
                                nc.sync.dma_start(wt[:],
                                                  wq_d[ts(dc, P), ts(mo, P)])
                                for qb in range(n_qb):
                                    nc.tensor.matmul(
                                        pss[qb][:], wt[:], XNQ[:, dc, ts(qb, QB)],
                                        start=(dc == 0), stop=(dc == n_dc - 1))
                            for qb in range(n_qb):
                                nc.vector.tensor_scalar_add(QT[:, mo, ts(qb, QB)],
                                                            pss[qb][:],
                                                            bq_t[:, mo:mo + 1])
                        # V: lhsT = XN chunk (tokens as M), rhs = Wv streamed
                        # per token-group (re-read n_tk/TG times)
                        NO = min(TB, Dm)
                        n_no = Dm // NO
                        TG = 4
                        for tg in range(0, n_tk, TG):
                            tos = range(tg, min(tg + TG, n_tk))
                            for no in range(n_no):
                                pss = [ps_mm.tile([P, NO], F32, tag="ps_mm",
                                                  name="ps_mm") for _ in tos]
                                for dc in range(n_dc):
                                    wvt = p_str.tile([P, NO], BF16, tag="wv")
                                    nc.sync.dma_start(
                                        wvt[:], wv_d[ts(dc, P), ts(no, NO)])
                                    for i, to in enumerate(tos):
                                        nc.tensor.matmul(
                                            pss[i][:], XN[:, dc, ts(to, P)],
                                            wvt[:],
                                            start=(dc == 0), stop=(dc == n_dc - 1))
                                for i, to in enumerate(tos):
                                    nc.vector.tensor_add(VT[:, to, ts(no, NO)],
                                                         pss[i][:],
                                                         bv_bc[:, ts(no, NO)])

                    if dbg:
                        for dc in range(n_dc):
                            nc.sync.dma_start(dbg_d["dKT"][ts(dc, P), :], KT[:, dc, :])
                            nc.sync.dma_start(dbg_d["dQT"][ts(dc, P), :], QT[:, dc, :])
                        for to in range(n_tk):
                            nc.sync.dma_start(dbg_d["dVT"][ts(to, P), :], VT[:, to, :])
                    # ---------- Phase 2: attention ----------
                    with tc.tile_pool(name="p_attn", bufs=1) as p_attn:
                        CT = p_attn.tile([P, n_dc, Q], BF16)
                        for dc in range(n_dc):
                            nc.sync.dma_start(XQ[:, dc, :], xqT_d[ts(dc, P), :])
                        with tc.tile_pool(name="p_exp", bufs=3) as p_exp, \
                             tc.tile_pool(name="p_rb", bufs=3) as p_rb, \
                             tc.tile_pool(name="ps_sc", bufs=2, space="PSUM") as ps_sc, \
                             tc.tile_pool(name="ps_ctx", bufs=1, space="PSUM") as ps_ctx, \
                             tc.tile_pool(name="ps_dn", bufs=2, space="PSUM") as ps_dn, \
                             tc.tile_pool(name="ps_wo", bufs=1, space="PSUM") as ps_wo, \
                             tc.tile_pool(name="p_wos", bufs=4) as p_wos:
                            for qq in range(n_qq):
                                qsl = ts(qq, QQ)
                                for hp in range(n_hp):
                                    exps = [p_exp.tile([P, n_tk, QQ], BF16,
                                                       tag="expT", name="expT")
                                            for _ in range(2)]
                                    # interleave the two heads' score matmuls:
                                    # they hit different PE row-strips (0/64)
                                    # and run concurrently in the array.
                                    # scores for 2 kc land in one 2-bank psum
                                    # tile so exp runs once per kc-pair.
                                    for kc in range(0, n_tk, 2):
                                        pss2 = [ps_sc.tile([P, 2, QQ], F32,
                                                           tag="ps_s", name="ps_s")
                                                for _ in range(2)]
                                        for j in range(2):
                                            for hi in range(2):
                                                r0 = hi * 64
                                                nc.tensor.matmul(
                                                    pss2[hi][:, j, :],
                                                    KT[r0:r0 + 64, hp,
                                                       ts(kc + j, P)],
                                                    QT[r0:r0 + 64, hp, qsl],
                                                    start=True, stop=True)
                                        for hi in range(2):
                                            nc.scalar.activation(
                                                exps[hi][:, kc:kc + 2, :],
                                                pss2[hi][:, :, :],
                                                AF.Exp, scale=0.125)
                                    rbcs = []
                                    dns = [ps_dn.tile([P, QQ], F32, tag="ps_d",
                                                      name="ps_d")
                                           for _ in range(2)]
                                    for kc in range(n_tk):
                                        for hi in range(2):
                                            nc.tensor.matmul(
                                                dns[hi][:], ones_f[:],
                                                exps[hi][:, kc, :],
                                                start=(kc == 0),
                                                stop=(kc == n_tk - 1))
                                    for hi in range(2):
                                        rbc_h = p_rb.tile([P, QQ], F32, tag="rbc",
                                                          name="rbc")
                                        nc.vector.reciprocal(rbc_h[:], dns[hi][:])
                                        rbcs.append(rbc_h)
                                    if dbg and hp == 0:
                                        nc.sync.dma_start(dbg_d["drbc"][0:64, qsl], rbcs[0][0:64, :])
                                        nc.sync.dma_start(dbg_d["drbc"][64:128, qsl], rbcs[1][64:128, :])
                                        for kc in range(n_tk):
                                            nc.sync.dma_start(
                                                dbg_d["dexp"][ts(kc, P), qsl],
                                                exps[0][:, kc, :])
                                    # interleaved ctx matmuls hit different PE
                                    # col-strips (0/64) -> concurrent
                                    ps_c = ps_ctx.tile([P, QQ], F32, tag="ps_c")
                                    for kc in range(n_tk):
                                        for hi in range(2):
                                            h = 2 * hp + hi
                                            nc.tensor.matmul(
                                                ps_c[hi * 64:hi * 64 + 64, :],
                                                VT[:, kc, ts(h, 64)],
                                                exps[hi][:, kc, :],
                                                start=(kc == 0),
                                                stop=(kc == n_tk - 1),
                                                tile_position=(0, hi * 64))
                                    for hi in range(2):
                                        r0 = hi * 64
                                        nc.vector.tensor_mul(
                                            CT[r0:r0 + 64, hp, qsl],
                                            ps_c[r0:r0 + 64, :],
                                            rbcs[hi][r0:r0 + 64, :])

                                # Wo + bias + residual for this q-block,
                                # overlapping the next q-block's attention
                                for mo in range(n_dc):
                                    ps_w = ps_wo.tile([P, QQ], F32, tag="ps_w")
                                    for dc in range(n_dc):
                                        wt = p_wos.tile([P, P], BF16, tag="wo")
                                        nc.sync.dma_start(
                                            wt[:], wo_d[ts(dc, P), ts(mo, P)])
                                        nc.tensor.matmul(
                                            ps_w[:], wt[:], CT[:, dc, qsl],
                                            start=(dc == 0), stop=(dc == n_dc - 1))
                                    nc.vector.tensor_add(XQ[:, mo, qsl],
                                                         ps_w[:],
                                                         XQ[:, mo, qsl])
                                    nc.vector.tensor_scalar_add(
                                        XQ[:, mo, qsl], XQ[:, mo, qsl],
                                        bo_t[:, mo:mo + 1])

                        if dbg:
                            for dc in range(n_dc):
                                nc.sync.dma_start(dbg_d["dCT"][ts(dc, P), :], CT[:, dc, :])

                if dbg:
                    for dc in range(n_dc):
                        nc.sync.dma_start(dbg_d["dh2"][ts(dc, P), :], XQ[:, dc, :])
                # ---------- Phase 3: LN2 + MLP ----------
                with tc.tile_pool(name="p_mlp", bufs=1) as p_mlp, \
                     tc.tile_pool(name="p_w1", bufs=3) as p_w1, \
                     tc.tile_pool(name="p_w2", bufs=3) as p_w2, \
                     tc.tile_pool(name="p_tmp2", bufs=2) as p_tmp2, \
                     tc.tile_pool(name="p_st2", bufs=1) as p_st2, \
                     tc.tile_pool(name="p_out", bufs=3) as p_out, \
                     tc.tile_pool(name="ps_st2", bufs=2, space="PSUM") as ps_st2, \
                     tc.tile_pool(name="ps_f", bufs=6, space="PSUM") as ps_f:

                    XN2 = p_mlp.tile([P, n_dc, Q], F32R)
                    _layernorm(nc, ones_h, ones_r, eps_t, p_mlp, p_tmp2, p_st2, ps_st2,
                               lambda dc: XQ[:, dc, :], n_dc, Q, QB,
                               g2_t, be2_t, lambda dc: XN2[:, dc, :])

                    # weight-outer loops so W1/W2 are read once; Y1 bf16 full-Q
                    Y1 = p_mlp.tile([P, n_mo, Q], BF16, tag="y1")
                    for mo in range(n_mo):
                        wt = p_w1.tile([P, n_dc, P], F32R, tag="w1")
                        nc.sync.dma_start(
                            wt[:],
                            w1_d[:, ts(mo, P)].rearrange("(c p) m -> p c m", p=P))
                        for qb in range(n_qb):
                            ps = ps_f.tile([P, QB], F32, tag="ps_f")
                            for dc in range(n_dc):
                                nc.tensor.matmul(ps[:], wt[:, dc, :],
                                                 XN2[:, dc, ts(qb, QB)],
                                                 start=(dc == 0),
                                                 stop=(dc == n_dc - 1))
                            nc.scalar.activation(Y1[:, mo, ts(qb, QB)], ps[:],
                                                 AF.Gelu, bias=b1_t[:, mo:mo + 1])
                    n_mh = max(1, n_mo // 2)
                    for mo2 in range(n_dc):
                        w2ts = []
                        for half in range(n_mo // n_mh):
                            wt = p_w2.tile([P, n_mh, P], BF16, tag="w2", name="w2")
                            nc.sync.dma_start(
                                wt[:],
                                w2_d[ts(half, n_mh * P), ts(mo2, P)]
                                .rearrange("(c p) m -> p c m", p=P))
                            w2ts.append(wt)
                        for qb in range(n_qb):
                            qsl = ts(qb, QB)
                            ps = ps_f.tile([P, QB], F32, tag="ps_f")
                            for kc in range(n_mo):
                                nc.tensor.matmul(ps[:],
                                                 w2ts[kc // n_mh][:, kc % n_mh, :],
                                                 Y1[:, kc, qsl],
                                                 start=(kc == 0),
                                                 stop=(kc == n_mo - 1))
                            ot = p_out.tile([P, QB], F32, tag="out")
                            nc.vector.tensor_add(ot[:], ps[:], XQ[:, mo2, qsl])
                            nc.vector.tensor_scalar_add(ot[:], ot[:],
                                                        b2_t[:, mo2:mo2 + 1])
                            nc.sync.dma_start(yT_d[ts(mo2, P), qsl], ot[:])
    nc.compile()
    return nc


_NC_CACHE = {}


def _get_nc(T, Q, Dm, Hh, Mlp, n_cores):
    key = (T, Q, Dm, Hh, Mlp, n_cores)
    if key not in _NC_CACHE:
        _NC_CACHE[key] = build_bass(T, Q, Dm, Hh, Mlp, n_cores)
    return _NC_CACHE[key]


def make_in_maps(inputs, n_cores):
    """Per-core input dicts for the (batch x seq-half) sharding."""
    x = np.asarray(inputs["x"], np.float32)
    Bq, Sq, Dq = x.shape
    Qtok = Sq * Bq // n_cores
    bf = ml_dtypes.bfloat16
    shared = {
        "g1": np.asarray(inputs["ln1_g"], np.float32),
        "be1": np.asarray(inputs["ln1_b"], np.float32),
        "g2": np.asarray(inputs["ln2_g"], np.float32),
        "be2": np.asarray(inputs["ln2_b"], np.float32),
        "wq16": np.asarray(inputs["Wq"], np.float32).astype(bf),
        "wk16": np.asarray(inputs["Wk"], np.float32).astype(bf),
        "wv16": np.asarray(inputs["Wv"], np.float32).astype(bf),
        "wo16": np.asarray(inputs["Wo"], np.float32).astype(bf),
        "w1r": np.asarray(inputs["W1"], np.float32),
        "w2r16": np.asarray(inputs["W2"], np.float32).astype(bf),
        "bq": np.asarray(inputs["bq"], np.float32),
        "bk": np.asarray(inputs["bk"], np.float32),
        "bv": np.asarray(inputs["bv"], np.float32),
        "bo": np.asarray(inputs["bo"], np.float32),
        "b1": np.asarray(inputs["b1"], np.float32),
        "b2": np.asarray(inputs["b2"], np.float32),
        "ones16": np.ones((P, 1), bf),
        "ones_r": np.ones((P, 1), np.float32),
    }
    in_maps = []
    for c in range(n_cores):
        b = c // (n_cores // Bq)
        qoff = (c % (n_cores // Bq)) * Qtok
        m = dict(shared)
        m["xT"] = np.ascontiguousarray(x[b].T)
        m["xqT"] = np.ascontiguousarray(x[b, qoff:qoff + Qtok].T)
        in_maps.append(m)
    return in_maps, Qtok


def kernel(**inputs):
    x = np.asarray(inputs["x"], np.float32)
    Bq, Sq, Dq = x.shape
    in_maps, Qtok = make_in_maps(inputs, N_CORES)
    nc = _get_nc(Sq, Qtok, Dq, H, MLP, N_CORES)
    res = run_bass_kernel_spmd(nc, in_maps, core_ids=list(range(N_CORES)))
    out = np.empty((Bq, Sq, Dq), np.float32)
    per_b = N_CORES // Bq
    for c in range(N_CORES):
        b = c // per_b
        qoff = (c % per_b) * Qtok
        out[b, qoff:qoff + Qtok, :] = res.results[c]["yT"].T
    return out



# revision 9
# speedup vs baseline: 1.5769x; 1.5769x over previous
"""Trainium2 Bass kernel for a dense transformer block (LN1 -> MHA -> LN2 -> MLP).

Sharding: 8 cores = (batch b in 0..3) x (sequence half in 0..1), zero
cross-core communication. Each core's input tokens are reordered on the host
so its 1024 query tokens are always tokens 0..1023 of its 2048-token view
(key/value order is irrelevant to attention), letting one SPMD program serve
every core and the query-side LN reuse the full-sequence LN output.

Precision: fp8e4m3 DoubleRow matmuls for QKV/O projections, ctx, and the MLP
(weights pre-scaled by power-of-2 factors on the host; descales fold into
existing bias/scale stages, so they cost nothing). Scores stay bf16.
LayerNorm gain/bias are folded into the following weights on the host
(mathematically exact), so the device LN is a pure (x-mu)*rstd normalize.

Softmax: exp(score - C) with a host-estimated shift C keeping exp outputs in
fp8 range; the denominator is produced by a ones-column appended to V inside
the ctx DoubleRow matmul (out partition 65), so it costs no extra PE time.
"""

import math
import sys

if '/opt/trn_rl_repo' not in sys.path:
    sys.path.insert(0, '/opt/trn_rl_repo')

import numpy as np
import ml_dtypes

import concourse.tile as tile
import concourse.mybir as mybir
from concourse import bacc
from concourse.bass import ts
from concourse.bass_utils import run_bass_kernel_spmd

P = 128
F32 = mybir.dt.float32
F32R = mybir.dt.float32r
BF16 = mybir.dt.bfloat16
F8 = mybir.dt.float8e4
AF = mybir.ActivationFunctionType
DR = mybir.MatmulPerfMode.DoubleRow
ALU = mybir.AluOpType
EPS = 1e-6

B, S, D, H, MLP = 4, 2048, 1024, 16, 4096
N_CORES = 8
NP_F8 = ml_dtypes.float8_e4m3


def build_bass(T, Q, Dm, Hh, Mlp, n_cores, scales):
    s_wq, s_wk, s_wv, s_wo, s_w1, s_w2, shift_c = scales
    dh = Dm // Hh
    assert dh == 64
    n_dc = Dm // P          # 8 feature chunks
    n_cj = n_dc // 2        # 4 DoubleRow k-pair steps over D
    n_tk = T // P           # 16 token chunks
    TB = 512
    n_tb = T // TB          # 4
    QQ = 512
    n_qq = Q // QQ          # 2
    n_mo = Mlp // P         # 32
    n_m2 = n_mo // 2        # 16 DoubleRow k-pair steps over MLP
    n_hp = Hh // 2          # 8 head pairs
    inv_d = 1.0 / Dm
    exp_scale = 0.125 / (s_wq * s_wk)
    c_wo = 1.0 / (s_wo * s_wv)
    inv_s1 = 1.0 / s_w1
    inv_s2 = 1.0 / s_w2

    nc = bacc.Bacc("TRN2", target_bir_lowering=False, debug=False,
                   enable_asserts=False, num_devices=n_cores)

    def din(name, shape, dt):
        return nc.dram_tensor(name, shape, dt, kind="ExternalInput").ap()

    xT_d = din("xT", (Dm, T), F32)
    wq_d, wk_d = din("wq8", (Dm, Dm), F8), din("wk8", (Dm, Dm), F8)
    wv_d, wo_d = din("wv8", (Dm, Dm), F8), din("wo8", (Dm, Dm), F8)
    w1_d = din("w18", (Dm, Mlp), F8)
    w2_d = din("w28", (Mlp, Dm), F8)
    bq_d, bk_d = din("bq", (Dm,), F32), din("bk", (Dm,), F32)
    bv_d, bo_d = din("bv", (Dm,), F32), din("bo", (Dm,), F32)
    b1_d, b2_d = din("b1", (Mlp,), F32), din("b2", (Dm,), F32)
    ones_d = din("ones32", (P, P), F32)
    yT_d = nc.dram_tensor("yT", (Dm, Q), F32, kind="ExternalOutput").ap()

    with tile.TileContext(nc) as tc:
        with tc.tile_pool(name="const", bufs=1) as constp:
            ones_fr = constp.tile([P, P], F32R)
            nc.sync.dma_start(ones_fr[:], ones_d[:, :].bitcast(F32R))
            ones_f = constp.tile([P, P], BF16)
            nc.vector.memset(ones_f[:], 1.0)
            eps_t = constp.tile([P, 1], F32)
            nc.vector.memset(eps_t[:], EPS)
            negc_t = constp.tile([P, 1], F32)
            nc.vector.memset(negc_t[:], -shift_c)

            def vec_tile(src, n, nm):
                t = constp.tile([P, n], F32, tag=nm, name=nm)
                nc.sync.dma_start(t[:], src.rearrange("(c p) -> p c", p=P))
                return t

            bq_t, bk_t = vec_tile(bq_d, n_dc, "bq"), vec_tile(bk_d, n_dc, "bk")
            bo_t, b2_t = vec_tile(bo_d, n_dc, "bo"), vec_tile(b2_d, n_dc, "b2")
            b1_t = vec_tile(b1_d, n_mo, "b1")
            # residual stream for the query tokens (bf16, copied from x chunks)
            with tc.tile_pool(name="p_xq", bufs=1) as p_xq:
                XQ = p_xq.tile([P, n_dc, Q], BF16)

                with tc.tile_pool(name="p_kv", bufs=1) as p_kv:
                    KT = p_kv.tile([P, n_dc, T], BF16)
                    QT = p_kv.tile([P, n_dc, Q], BF16)
                    VT = p_kv.tile([P, n_tk, Hh, 66], F8)
                    nc.gpsimd.memset(VT[:, :, :, 64:66], 1.0)

                    # ---------- Phase A: LN1 + QKV projections ----------
                    with tc.tile_pool(name="p_xn8", bufs=1) as p_xn8, \
                         tc.tile_pool(name="p_x", bufs=2) as p_x, \
                         tc.tile_pool(name="p_t", bufs=2) as p_t, \
                         tc.tile_pool(name="p_w", bufs=3) as p_w, \
                         tc.tile_pool(name="p_wv", bufs=2) as p_wv, \
                         tc.tile_pool(name="ps_st", bufs=2, space="PSUM") as ps_st, \
                         tc.tile_pool(name="ps_p", bufs=4, space="PSUM") as ps_p:

                        XN8 = p_xn8.tile([P, n_dc, T], F8)
                        bv_row = p_xn8.tile([1, Dm], F32)
                        nc.sync.dma_start(bv_row[:, :], bv_d[None, :])
                        bv_bc = p_xn8.tile([P, Dm], F32)
                        nc.gpsimd.partition_broadcast(bv_bc[:], bv_row[:])
                        for tb in range(n_tb):
                            tsl = ts(tb, TB)
                            xc = p_x.tile([P, n_dc, TB], F32R, tag="xc")
                            for dc in range(n_dc):
                                nc.sync.dma_start(
                                    xc[:, dc, :],
                                    xT_d[ts(dc, P), tsl].bitcast(F32R))
                            st = ps_st.tile([P, 2, TB], F32, tag="st")
                            for dc in range(n_dc):
                                nc.tensor.matmul(st[:, 0, :], ones_fr[:],
                                                 xc[:, dc, :],
                                                 start=(dc == 0),
                                                 stop=(dc == n_dc - 1))
                                xsq = p_t.tile([P, TB], BF16, tag="xsq")
                                nc.scalar.activation(xsq[:],
                                                     xc[:, dc, :].bitcast(F32),
                                                     AF.Square)
                                nc.tensor.matmul(st[:, 1, :], ones_f[:],
                                                 xsq[:],
                                                 start=(dc == 0),
                                                 stop=(dc == n_dc - 1))
                                if tb < Q // TB:
                                    nc.vector.tensor_copy(
                                        XQ[:, dc, tsl],
                                        xc[:, dc, :].bitcast(F32))
                            mbc = p_t.tile([P, TB], F32, tag="mbc")
                            nc.vector.tensor_scalar_mul(mbc[:], st[:, 0, :], inv_d)
                            var = p_t.tile([P, TB], F32, tag="var")
                            nc.vector.tensor_scalar_mul(var[:], st[:, 1, :], inv_d)
                            m2 = p_t.tile([P, TB], F32, tag="m2")
                            nc.vector.tensor_mul(m2[:], mbc[:], mbc[:])
                            nc.vector.tensor_sub(var[:], var[:], m2[:])
                            lnv = p_t.tile([P, TB], F32, tag="lnv")
                            nc.scalar.activation(lnv[:], var[:], AF.Ln,
                                                 bias=eps_t[:, 0:1])
                            rstd = p_t.tile([P, TB], F32, tag="rstd")
                            nc.scalar.activation(rstd[:], lnv[:], AF.Exp,
                                                 scale=-0.5)
                            for dc in range(n_dc):
                                t0 = p_t.tile([P, TB], F32, tag="t0")
                                nc.vector.tensor_sub(t0[:],
                                                     xc[:, dc, :].bitcast(F32),
                                                     mbc[:])
                                nc.gpsimd.tensor_mul(XN8[:, dc, tsl],
                                                     t0[:], rstd[:])

                        def q_proj(mo):
                            wt = p_w.tile([P, n_cj, 2, P], F8, tag="wq")
                            nc.sync.dma_start(
                                wt[:],
                                wq_d[:, ts(mo, P)]
                                .rearrange("(c j p) m -> p c j m", j=2, p=P))
                            for qb in range(n_qq):
                                ps = ps_p.tile([P, QQ], F32, tag="pp")
                                for cj in range(n_cj):
                                    nc.tensor.matmul(
                                        ps[:], wt[:, cj, :, :],
                                        XN8[:, 2 * cj:2 * cj + 2, ts(qb, QQ)],
                                        start=(cj == 0), stop=(cj == n_cj - 1),
                                        perf_mode=DR)
                                nc.vector.tensor_scalar_add(
                                    QT[:, mo, ts(qb, QQ)], ps[:],
                                    bq_t[:, mo:mo + 1])

                        def k_proj(mo):
                            wt = p_w.tile([P, n_cj, 2, P], F8, tag="wq")
                            nc.sync.dma_start(
                                wt[:],
                                wk_d[:, ts(mo, P)]
                                .rearrange("(c j p) m -> p c j m", j=2, p=P))
                            for tb in range(n_tb):
                                ps = ps_p.tile([P, TB], F32, tag="pp")
                                for cj in range(n_cj):
                                    nc.tensor.matmul(
                                        ps[:], wt[:, cj, :, :],
                                        XN8[:, 2 * cj:2 * cj + 2, ts(tb, TB)],
                                        start=(cj == 0), stop=(cj == n_cj - 1),
                                        perf_mode=DR)
                                nc.vector.tensor_scalar_add(
                                    KT[:, mo, ts(tb, TB)], ps[:],
                                    bk_t[:, mo:mo + 1])

                        def v_proj(no):
                            NO = 512
                            wt = p_wv.tile([P, n_cj, 2, NO], F8, tag="wv")
                            nc.sync.dma_start(
                                wt[:],
                                wv_d[:, ts(no, NO)]
                                .rearrange("(c j p) m -> p c j m", j=2, p=P))
                            for to in range(n_tk):
                                ps = ps_p.tile([P, NO], F32, tag="pp")
                                for cj in range(n_cj):
                                    nc.tensor.matmul(
                                        ps[:],
                                        XN8[:, 2 * cj:2 * cj + 2, ts(to, P)],
                                        wt[:, cj, :, :],
                                        start=(cj == 0), stop=(cj == n_cj - 1),
                                        perf_mode=DR)
                                nc.vector.tensor_add(
                                    VT[:, to, 8 * no:8 * no + 8, 0:64],
                                    ps[:], bv_bc[:, ts(no, NO)])

                        for mo in range(4):
                            k_proj(mo)
                            q_proj(mo)
                        v_proj(0)
                        for mo in range(4, n_dc):
                            k_proj(mo)
                            q_proj(mo)
                        v_proj(1)

                    # ---------- Phase B: attention + Wo, MLP interleaved ----
                    with tc.tile_pool(name="p_ct", bufs=1) as p_ct, \
                         tc.tile_pool(name="p_mlp", bufs=1) as p_mlp, \
                         tc.tile_pool(name="p_exp", bufs=3) as p_exp, \
                         tc.tile_pool(name="p_rb", bufs=2) as p_rb, \
                         tc.tile_pool(name="p_t2", bufs=2) as p_t2, \
                         tc.tile_pool(name="p_wos", bufs=2) as p_wos, \
                         tc.tile_pool(name="p_w1", bufs=3) as p_w1, \
                         tc.tile_pool(name="p_w2", bufs=2) as p_w2, \
                         tc.tile_pool(name="p_y1", bufs=1) as p_y1, \
                         tc.tile_pool(name="p_out", bufs=2) as p_out, \
                         tc.tile_pool(name="ps_sc", bufs=2, space="PSUM") as ps_sc, \
                         tc.tile_pool(name="ps_ctx", bufs=2, space="PSUM") as ps_ctx, \
                         tc.tile_pool(name="ps_sh", bufs=2, space="PSUM") as ps_sh:

                        CT8 = p_ct.tile([P, n_dc, Q], F8)
                        XN2 = p_mlp.tile([P, n_dc, Q], F8)

                        def attn_block(qq, hp):
                            qsl = ts(qq, QQ)
                            exps = [p_exp.tile([P, n_tk, QQ], F8,
                                               tag="expT", name="expT")
                                    for _ in range(2)]
                            for kcp in range(n_tk // 2):
                                for hi in range(2):
                                    r0 = hi * 64
                                    pss = ps_sc.tile([P, 2, QQ], F32,
                                                     tag="ps_s", name="ps_s")
                                    for j in range(2):
                                        nc.tensor.matmul(
                                            pss[:, j, :],
                                            KT[r0:r0 + 64, hp,
                                               ts(2 * kcp + j, P)],
                                            QT[r0:r0 + 64, hp, qsl],
                                            start=True, stop=True)
                                    nc.scalar.activation(
                                        exps[hi][:, 2 * kcp:2 * kcp + 2, :],
                                        pss[:, :, :], AF.Exp,
                                        scale=exp_scale, bias=negc_t[:, 0:1])
                            pcs = [ps_ctx.tile([65, QQ], F32, tag="ps_c",
                                               name="ps_c")
                                   for _ in range(2)]
                            for kcp in range(n_tk // 2):
                                for hi in range(2):
                                    h = 2 * hp + hi
                                    nc.tensor.matmul(
                                        pcs[hi][:, :],
                                        VT[:, 2 * kcp:2 * kcp + 2, h, 0:65],
                                        exps[hi][:, 2 * kcp:2 * kcp + 2, :],
                                        start=(kcp == 0),
                                        stop=(kcp == n_tk // 2 - 1),
                                        perf_mode=DR)
                            for hi in range(2):
                                r0 = hi * 64
                                rb1 = p_rb.tile([1, QQ], F32, tag="rb1")
                                nc.vector.reciprocal(rb1[:], pcs[hi][64:65, :])
                                rbb = p_rb.tile([64, QQ], F32, tag="rbb")
                                nc.gpsimd.partition_broadcast(rbb[:], rb1[:])
                                nc.vector.tensor_mul(CT8[r0:r0 + 64, hp, qsl],
                                                     pcs[hi][0:64, :], rbb[:])

                        def wo_block(qq):
                            qsl = ts(qq, QQ)
                            for mo in range(n_dc):
                                wt = p_wos.tile([P, n_cj, 2, P], F8, tag="wo")
                                nc.sync.dma_start(
                                    wt[:],
                                    wo_d[:, ts(mo, P)]
                                    .rearrange("(c j p) m -> p c j m", j=2, p=P))
                                ps = ps_sh.tile([P, QQ], F32, tag="sh")
                                for cj in range(n_cj):
                                    nc.tensor.matmul(
                                        ps[:], wt[:, cj, :, :],
                                        CT8[:, 2 * cj:2 * cj + 2, qsl],
                                        start=(cj == 0), stop=(cj == n_cj - 1),
                                        perf_mode=DR)
                                tw = p_out.tile([P, QQ], F32, tag="ot")
                                nc.vector.tensor_scalar(
                                    tw[:], ps[:], c_wo, bo_t[:, mo:mo + 1],
                                    op0=ALU.mult, op1=ALU.add)
                                nc.vector.tensor_add(XQ[:, mo, qsl],
                                                     tw[:],
                                                     XQ[:, mo, qsl])

                        def mlp_block(qq):
                            qsl = ts(qq, QQ)
                            st2 = ps_sc.tile([P, 2, QQ], F32, tag="ps_s",
                                             name="ps_s")
                            for dc in range(n_dc):
                                nc.tensor.matmul(st2[:, 0, :], ones_f[:],
                                                 XQ[:, dc, qsl],
                                                 start=(dc == 0),
                                                 stop=(dc == n_dc - 1))
                                sq = p_t2.tile([P, QQ], BF16, tag="sq2")
                                nc.scalar.activation(sq[:], XQ[:, dc, qsl],
                                                     AF.Square)
                                nc.tensor.matmul(st2[:, 1, :], ones_f[:], sq[:],
                                                 start=(dc == 0),
                                                 stop=(dc == n_dc - 1))
                            mbc = p_t2.tile([P, QQ], F32, tag="mbc2")
                            nc.vector.tensor_scalar_mul(mbc[:], st2[:, 0, :], inv_d)
                            var = p_t2.tile([P, QQ], F32, tag="var2")
                            nc.vector.tensor_scalar_mul(var[:], st2[:, 1, :], inv_d)
                            m2 = p_t2.tile([P, QQ], F32, tag="m22")
                            nc.vector.tensor_mul(m2[:], mbc[:], mbc[:])
                            nc.vector.tensor_sub(var[:], var[:], m2[:])
                            lnv = p_t2.tile([P, QQ], F32, tag="lnv2")
                            nc.scalar.activation(lnv[:], var[:], AF.Ln,
                                                 bias=eps_t[:, 0:1])
                            rstd = p_t2.tile([P, QQ], F32, tag="rstd2")
                            nc.scalar.activation(rstd[:], lnv[:], AF.Exp,
                                                 scale=-0.5)
                            for dc in range(n_dc):
                                t0 = p_t2.tile([P, QQ], F32, tag="t02")
                                nc.vector.tensor_sub(t0[:],
                                                     XQ[:, dc, qsl],
                                                     mbc[:])
                                nc.gpsimd.tensor_mul(XN2[:, dc, qsl],
                                                     t0[:], rstd[:])
                            # fc1
                            Y1 = p_y1.tile([P, n_mo, QQ], F8, tag="y1",
                                           name="y1")
                            for mo in range(n_mo):
                                wt = p_w1.tile([P, n_cj, 2, P], F8, tag="w1")
                                nc.sync.dma_start(
                                    wt[:],
                                    w1_d[:, ts(mo, P)]
                                    .rearrange("(c j p) m -> p c j m", j=2, p=P))
                                ps = ps_sh.tile([P, QQ], F32, tag="sh")
                                for cj in range(n_cj):
                                    nc.tensor.matmul(
                                        ps[:], wt[:, cj, :, :],
                                        XN2[:, 2 * cj:2 * cj + 2, qsl],
                                        start=(cj == 0), stop=(cj == n_cj - 1),
                                        perf_mode=DR)
                                nc.scalar.activation(Y1[:, mo, :], ps[:],
                                                     AF.Gelu,
                                                     bias=b1_t[:, mo:mo + 1],
                                                     scale=inv_s1)
                            # fc2
                            for mo2 in range(n_dc):
                                wt = p_w2.tile([P, n_m2, 2, P], F8, tag="w2")
                                nc.sync.dma_start(
                                    wt[:],
                                    w2_d[:, ts(mo2, P)]
                                    .rearrange("(c j p) m -> p c j m", j=2, p=P))
                                ps = ps_sh.tile([P, QQ], F32, tag="sh")
                                for cj in range(n_m2):
                                    nc.tensor.matmul(
                                        ps[:], wt[:, cj, :, :],
                                        Y1[:, 2 * cj:2 * cj + 2, :],
                                        start=(cj == 0), stop=(cj == n_m2 - 1),
                                        perf_mode=DR)
                                ot = p_out.tile([P, QQ], F32, tag="ot")
                                nc.vector.tensor_scalar(
                                    ot[:], ps[:], inv_s2, b2_t[:, mo2:mo2 + 1],
                                    op0=ALU.mult, op1=ALU.add)
                                nc.vector.tensor_add(ot[:], ot[:],
                                                     XQ[:, mo2, qsl])
                                nc.sync.dma_start(yT_d[ts(mo2, P), qsl], ot[:])

                        for qq in range(n_qq):
                            for hp in range(n_hp):
                                attn_block(qq, hp)
                            wo_block(qq)
                            mlp_block(qq)
    nc.compile()
    return nc


_NC_CACHE = {}


def _get_nc(T, Q, Dm, Hh, Mlp, n_cores,
            scales=(16.0, 16.0, 16.0, 16.0, 16.0, 16.0, 3.5)):
    key = (T, Q, Dm, Hh, Mlp, n_cores, tuple(scales))
    if key not in _NC_CACHE:
        _NC_CACHE[key] = build_bass(T, Q, Dm, Hh, Mlp, n_cores, scales)
    return _NC_CACHE[key]


def _pow2_scale(absmax, target=128.0):
    a = float(absmax)
    if not np.isfinite(a) or a <= 0:
        return 1.0
    return float(2.0 ** math.floor(math.log2(target / a)))


def prepare(inputs):
    """Host-side prep: LN folding, fp8 quantization, per-core input maps."""
    f = lambda k: np.asarray(inputs[k], np.float32)
    x = f("x")
    Bq, Sq, Dq = x.shape
    Qtok = Sq // 2
    g1, b1ln = f("ln1_g"), f("ln1_b")
    g2, b2ln = f("ln2_g"), f("ln2_b")
    Wq, Wk, Wv, Wo = f("Wq"), f("Wk"), f("Wv"), f("Wo")
    W1, W2 = f("W1"), f("W2")
    bq, bk, bv, bo = f("bq"), f("bk"), f("bv"), f("bo")
    b1, b2 = f("b1"), f("b2")

    # fold LN1 gain/bias into QKV, LN2 gain/bias into W1 (exact)
    Wq_e = g1[:, None] * Wq
    Wk_e = g1[:, None] * Wk
    Wv_e = g1[:, None] * Wv
    bq_e = bq + b1ln @ Wq
    bk_e = bk + b1ln @ Wk
    bv_e = bv + b1ln @ Wv
    W1_e = g2[:, None] * W1
    b1_e = b1 + b2ln @ W1

    s_wq = _pow2_scale(np.abs(Wq_e).max())
    s_wk = _pow2_scale(np.abs(Wk_e).max())
    # V result is stored in fp8 still scaled by s_wv: bound both weight and
    # activation range (sigma of v_j ~ col norm of Wv_e, x is LN'd)
    vcol = np.sqrt((Wv_e ** 2).sum(0))
    vmag = max(float((vcol * 8).max()), float(np.abs(bv_e).max() * 4), 1e-6)
    s_wv = min(_pow2_scale(np.abs(Wv_e).max()),
               _pow2_scale(vmag, target=200.0))
    s_wo = _pow2_scale(np.abs(Wo).max())
    s_w1 = _pow2_scale(np.abs(W1_e).max())
    s_w2 = _pow2_scale(np.abs(W2).max())

    # estimate max attention score for the exp shift C (sampled)
    mu = x.mean(-1, keepdims=True)
    va = x.var(-1, keepdims=True)
    xn_h = (x - mu) / np.sqrt(va + EPS)
    qi = xn_h[:, ::89][:, :16].reshape(-1, Dq)
    ki = xn_h[:, ::13][:, :128].reshape(-1, Dq)
    qp = (qi @ Wq_e + bq_e).reshape(Bq, -1, H, Dq // H)
    kp = (ki @ Wk_e + bk_e).reshape(Bq, -1, H, Dq // H)
    sc = np.einsum("bqhd,bkhd->bhqk", qp, kp) / np.sqrt(Dq // H)
    shift_c = float(sc.max() + 2.0 * sc.std() - math.log(32.0))

    scales = (s_wq, s_wk, s_wv, s_wo, s_w1, s_w2, shift_c)
    nc = _get_nc(Sq, Qtok, Dq, H, MLP, N_CORES, scales)

    shared = {
        "wq8": (Wq_e * s_wq).astype(NP_F8),
        "wk8": (Wk_e * s_wk).astype(NP_F8),
        "wv8": (Wv_e * s_wv).astype(NP_F8),
        "wo8": (Wo * s_wo).astype(NP_F8),
        "w18": (W1_e * s_w1).astype(NP_F8),
        "w28": (W2 * s_w2).astype(NP_F8),
        "bq": (bq_e * s_wq).astype(np.float32),
        "bk": (bk_e * s_wk).astype(np.float32),
        "bv": (bv_e * s_wv).astype(np.float32),
        "bo": bo.astype(np.float32),
        "b1": b1_e.astype(np.float32),
        "b2": b2.astype(np.float32),
        "ones32": np.ones((P, P), np.float32),
    }
    in_maps = []
    for c in range(N_CORES):
        b = c // 2
        half = c % 2
        xb = x[b]
        xr = np.concatenate(
            [xb[half * Qtok:(half + 1) * Qtok],
             xb[(1 - half) * Qtok:(2 - half) * Qtok]], axis=0)
        m = dict(shared)
        m["xT"] = np.ascontiguousarray(xr.T)
        in_maps.append(m)
    return nc, in_maps, Qtok


def unshard(res, Bq, Sq, Dq, Qtok):
    out = np.empty((Bq, Sq, Dq), np.float32)
    for c in range(N_CORES):
        b = c // 2
        half = c % 2
        out[b, half * Qtok:(half + 1) * Qtok, :] = res.results[c]["yT"].T
    return out


def kernel(**inputs):
    x = np.asarray(inputs["x"], np.float32)
    Bq, Sq, Dq = x.shape
    nc, in_maps, Qtok = prepare(inputs)
    res = run_bass_kernel_spmd(nc, in_maps, core_ids=list(range(N_CORES)))
    return unshard(res, Bq, Sq, Dq, Qtok)


# revision 10
# speedup vs baseline: 1.7010x; 1.0786x over previous
"""Trainium2 Bass kernel for a dense transformer block (LN1 -> MHA -> LN2 -> MLP).

Sharding: 8 cores = (batch b in 0..3) x (sequence half in 0..1), zero
cross-core communication. Each core's input tokens are reordered on the host
so its 1024 query tokens are always tokens 0..1023 of its 2048-token view
(key/value order is irrelevant to attention), letting one SPMD program serve
every core and the query-side LN reuse the full-sequence LN output.

Precision: fp8e4m3 DoubleRow matmuls for QKV/O projections, ctx, and the MLP
(weights pre-scaled by power-of-2 factors on the host; descales fold into
existing bias/scale stages, so they cost nothing). Scores stay bf16.
LayerNorm gain/bias are folded into the following weights on the host
(mathematically exact), so the device LN is a pure (x-mu)*rstd normalize.

Softmax: exp(score - C) with a host-estimated shift C keeping exp outputs in
fp8 range; the denominator is produced by a ones-column appended to V inside
the ctx DoubleRow matmul (out partition 65), so it costs no extra PE time.

Schedule: attention for the first head-pairs is emitted between the two
projection groups so the Act engine's exp stream starts as early as possible;
the MLP of each query half is chunk-interleaved into the next half's
attention so gelus stay clustered (minimizing activation-table reloads) while
PE fills Act-bound stretches. LN2's rstd is computed entirely on DVE
(reciprocal_approx_fast + Newton) to avoid sqrt-table loads mid-stream.
"""

import math
import sys

if '/opt/trn_rl_repo' not in sys.path:
    sys.path.insert(0, '/opt/trn_rl_repo')

import numpy as np
import ml_dtypes

import concourse.tile as tile
import concourse.mybir as mybir
from concourse import bacc
from concourse.bass import ts
from concourse.bass_utils import run_bass_kernel_spmd

P = 128
F32 = mybir.dt.float32
F32R = mybir.dt.float32r
BF16 = mybir.dt.bfloat16
F8 = mybir.dt.float8e4
AF = mybir.ActivationFunctionType
DR = mybir.MatmulPerfMode.DoubleRow
ALU = mybir.AluOpType
EPS = 1e-6

B, S, D, H, MLP = 4, 2048, 1024, 16, 4096
N_CORES = 8
NP_F8 = ml_dtypes.float8_e4m3


def build_bass(T, Q, Dm, Hh, Mlp, n_cores, scales):
    s_wq, s_wk, s_wv, s_wo, s_w1, s_w2, shift_c = scales
    dh = Dm // Hh
    assert dh == 64
    n_dc = Dm // P          # 8 feature chunks
    n_cj = n_dc // 2        # 4 DoubleRow k-pair steps over D
    n_tk = T // P           # 16 token chunks
    TB = 512
    n_tb = T // TB          # 4
    QQ = 512
    n_qq = Q // QQ          # 2
    n_mo = Mlp // P         # 32
    n_m2 = n_mo // 2        # 16 DoubleRow k-pair steps over MLP
    n_hp = Hh // 2          # 8 head pairs
    inv_d = 1.0 / Dm
    exp_scale = 0.125 / (s_wq * s_wk)
    c_wo = 1.0 / (s_wo * s_wv)
    inv_s1 = 1.0 / s_w1
    inv_s2 = 1.0 / s_w2

    nc = bacc.Bacc("TRN2", target_bir_lowering=False, debug=False,
                   enable_asserts=False, num_devices=n_cores)

    def din(name, shape, dt):
        return nc.dram_tensor(name, shape, dt, kind="ExternalInput").ap()

    xT_d = din("xT", (Dm, T), F32)
    wq_d, wk_d = din("wq8", (Dm, Dm), F8), din("wk8", (Dm, Dm), F8)
    wv_d, wo_d = din("wv8", (Dm, Dm), F8), din("wo8", (Dm, Dm), F8)
    w1_d = din("w18", (Dm, Mlp), F8)
    w2_d = din("w28", (Mlp, Dm), F8)
    bq_d, bk_d = din("bq", (Dm,), F32), din("bk", (Dm,), F32)
    bv_d, bo_d = din("bv16", (Dm,), BF16), din("bo", (Dm,), F32)
    b1_d, b2_d = din("b1", (Mlp,), F32), din("b2", (Dm,), F32)
    ones_d = din("ones32", (P, P), F32)
    yT_d = nc.dram_tensor("yT", (Dm, Q), F32, kind="ExternalOutput").ap()

    with tile.TileContext(nc) as tc, \
         tc.tile_pool(name="const", bufs=1) as constp, \
         tc.tile_pool(name="p_res", bufs=1) as p_res, \
         tc.tile_pool(name="p_kv", bufs=1) as p_kv, \
         tc.tile_pool(name="p_exp", bufs=3) as p_exp, \
         tc.tile_pool(name="p_rb", bufs=2) as p_rb, \
         tc.tile_pool(name="p_ct", bufs=1) as p_ct, \
         tc.tile_pool(name="ps_sc", bufs=2, space="PSUM") as ps_sc, \
         tc.tile_pool(name="ps_ctx", bufs=2, space="PSUM") as ps_ctx, \
         tc.tile_pool(name="ps_sh", bufs=2, space="PSUM") as ps_sh:

        ones_fr = constp.tile([P, P], F32R)
        nc.sync.dma_start(ones_fr[:], ones_d[:, :].bitcast(F32R))
        ones_f = constp.tile([P, P], BF16)
        nc.vector.memset(ones_f[:], 1.0)
        eps_t = constp.tile([P, 1], F32)
        nc.vector.memset(eps_t[:], EPS)
        negc_t = constp.tile([P, 1], F32)
        nc.vector.memset(negc_t[:], -shift_c)

        def vec_tile(src, n, nm):
            t = constp.tile([P, n], F32, tag=nm, name=nm)
            nc.sync.dma_start(t[:], src.rearrange("(c p) -> p c", p=P))
            return t

        bq_t, bk_t = vec_tile(bq_d, n_dc, "bq"), vec_tile(bk_d, n_dc, "bk")
        bo_t, b2_t = vec_tile(bo_d, n_dc, "bo"), vec_tile(b2_d, n_dc, "b2")
        b1_t = vec_tile(b1_d, n_mo, "b1")

        XQ = p_res.tile([P, n_dc, Q], BF16)       # residual stream (bf16)
        KT = p_kv.tile([P, n_dc, T], BF16)
        QT = p_kv.tile([P, n_dc, Q], BF16)
        VT = p_kv.tile([P, n_tk, Hh, 66], F8)
        nc.gpsimd.memset(VT[:, :, :, 64:66], 1.0)
        CT8 = p_ct.tile([P, n_dc, Q], F8)

        def attn_block(qq, hp):
            qsl = ts(qq, QQ)
            exps = [p_exp.tile([P, n_tk, QQ], F8, tag="expT", name="expT")
                    for _ in range(2)]
            for kcp in range(n_tk // 2):
                for hi in range(2):
                    r0 = hi * 64
                    pss = ps_sc.tile([P, 2, QQ], F32, tag="ps_s", name="ps_s")
                    for j in range(2):
                        nc.tensor.matmul(
                            pss[:, j, :],
                            KT[r0:r0 + 64, hp, ts(2 * kcp + j, P)],
                            QT[r0:r0 + 64, hp, qsl],
                            start=True, stop=True)
                    nc.scalar.activation(
                        exps[hi][:, 2 * kcp:2 * kcp + 2, :],
                        pss[:, :, :], AF.Exp,
                        scale=exp_scale, bias=negc_t[:, 0:1])
            pcs = [ps_ctx.tile([65, QQ], F32, tag="ps_c", name="ps_c")
                   for _ in range(2)]
            for kcp in range(n_tk // 2):
                for hi in range(2):
                    h = 2 * hp + hi
                    nc.tensor.matmul(
                        pcs[hi][:, :],
                        VT[:, 2 * kcp:2 * kcp + 2, h, 0:65],
                        exps[hi][:, 2 * kcp:2 * kcp + 2, :],
                        start=(kcp == 0), stop=(kcp == n_tk // 2 - 1),
                        perf_mode=DR)
            for hi in range(2):
                r0 = hi * 64
                rb1 = p_rb.tile([1, QQ], F32, tag="rb1")
                nc.vector.reciprocal_approx_fast(rb1[:], pcs[hi][64:65, :])
                rbb = p_rb.tile([64, QQ], F32, tag="rbb")
                nc.gpsimd.partition_broadcast(rbb[:], rb1[:])
                nc.vector.tensor_mul(CT8[r0:r0 + 64, hp, qsl],
                                     pcs[hi][0:64, :], rbb[:])

        # ================= Phase A: LN1 + projections (+early attn) ========
        with tc.tile_pool(name="p_xn8", bufs=1) as p_xn8, \
             tc.tile_pool(name="p_x", bufs=2) as p_x, \
             tc.tile_pool(name="p_t", bufs=2) as p_t, \
             tc.tile_pool(name="p_w", bufs=3) as p_w, \
             tc.tile_pool(name="p_wv", bufs=2) as p_wv:

            XN8 = p_xn8.tile([P, n_dc, T], F8)
            bv_row = p_xn8.tile([1, Dm], BF16)
            nc.sync.dma_start(bv_row[:, :], bv_d[None, :])
            bv_bc = p_xn8.tile([P, Dm], BF16)
            nc.gpsimd.partition_broadcast(bv_bc[:], bv_row[:])

            for tb in range(n_tb):
                tsl = ts(tb, TB)
                xc = p_x.tile([P, n_dc, TB], F32R, tag="xc")
                for dc in range(n_dc):
                    nc.sync.dma_start(xc[:, dc, :],
                                      xT_d[ts(dc, P), tsl].bitcast(F32R))
                st = ps_sc.tile([P, 2, TB], F32, tag="ps_s", name="ps_s")
                for dc in range(n_dc):
                    nc.tensor.matmul(st[:, 0, :], ones_fr[:], xc[:, dc, :],
                                     start=(dc == 0), stop=(dc == n_dc - 1))
                    xsq = p_t.tile([P, TB], BF16, tag="xsq")
                    nc.scalar.activation(xsq[:], xc[:, dc, :].bitcast(F32),
                                         AF.Square)
                    nc.tensor.matmul(st[:, 1, :], ones_f[:], xsq[:],
                                     start=(dc == 0), stop=(dc == n_dc - 1))
                    if tb < Q // TB:
                        nc.vector.tensor_copy(XQ[:, dc, tsl],
                                              xc[:, dc, :].bitcast(F32))
                mbc = p_t.tile([P, TB], F32, tag="mbc")
                nc.vector.tensor_scalar_mul(mbc[:], st[:, 0, :], inv_d)
                var = p_t.tile([P, TB], F32, tag="var")
                nc.vector.tensor_scalar(var[:], st[:, 1, :], inv_d, EPS,
                                        op0=ALU.mult, op1=ALU.add)
                m2 = p_t.tile([P, TB], F32, tag="tn")
                nc.vector.tensor_mul(m2[:], mbc[:], mbc[:])
                nc.vector.tensor_sub(var[:], var[:], m2[:])
                std = p_t.tile([P, TB], F32, tag="stdt")
                nc.scalar.activation(std[:], var[:], AF.Sqrt)
                rstd = p_t.tile([P, TB], F32, tag="rstd")
                nc.vector.reciprocal(rstd[:], std[:])
                for dc in range(n_dc):
                    t0 = p_t.tile([P, TB], F32, tag="tn")
                    nc.vector.tensor_sub(t0[:], xc[:, dc, :].bitcast(F32),
                                         mbc[:])
                    nc.gpsimd.tensor_mul(XN8[:, dc, tsl], t0[:], rstd[:])

            def q_proj(mo):
                wt = p_w.tile([P, n_cj, 2, P], F8, tag="wq")
                nc.sync.dma_start(
                    wt[:],
                    wq_d[:, ts(mo, P)]
                    .rearrange("(c j p) m -> p c j m", j=2, p=P))
                for qb in range(n_qq):
                    ps = ps_sh.tile([P, QQ], F32, tag="sh")
                    for cj in range(n_cj):
                        nc.tensor.matmul(
                            ps[:], wt[:, cj, :, :],
                            XN8[:, 2 * cj:2 * cj + 2, ts(qb, QQ)],
                            start=(cj == 0), stop=(cj == n_cj - 1),
                            perf_mode=DR)
                    nc.vector.tensor_scalar_add(QT[:, mo, ts(qb, QQ)], ps[:],
                                                bq_t[:, mo:mo + 1])

            def k_proj(mo):
                wt = p_w.tile([P, n_cj, 2, P], F8, tag="wq")
                nc.sync.dma_start(
                    wt[:],
                    wk_d[:, ts(mo, P)]
                    .rearrange("(c j p) m -> p c j m", j=2, p=P))
                for tb in range(n_tb):
                    ps = ps_sh.tile([P, TB], F32, tag="sh")
                    for cj in range(n_cj):
                        nc.tensor.matmul(
                            ps[:], wt[:, cj, :, :],
                            XN8[:, 2 * cj:2 * cj + 2, ts(tb, TB)],
                            start=(cj == 0), stop=(cj == n_cj - 1),
                            perf_mode=DR)
                    nc.vector.tensor_scalar_add(KT[:, mo, ts(tb, TB)], ps[:],
                                                bk_t[:, mo:mo + 1])

            def v_proj(no):
                NO = 512
                wt = p_wv.tile([P, n_cj, 2, NO], F8, tag="wv")
                nc.sync.dma_start(
                    wt[:],
                    wv_d[:, ts(no, NO)]
                    .rearrange("(c j p) m -> p c j m", j=2, p=P))
                for to in range(n_tk):
                    ps = ps_sh.tile([P, NO], F32, tag="sh")
                    for cj in range(n_cj):
                        nc.tensor.matmul(
                            ps[:], XN8[:, 2 * cj:2 * cj + 2, ts(to, P)],
                            wt[:, cj, :, :],
                            start=(cj == 0), stop=(cj == n_cj - 1),
                            perf_mode=DR)
                    nc.vector.tensor_add(VT[:, to, 8 * no:8 * no + 8, 0:64],
                                         ps[:], bv_bc[:, ts(no, NO)])

            for mo in range(4):
                k_proj(mo)
                q_proj(mo)
            v_proj(0)
            for hp in range(2):
                attn_block(0, hp)
            for mo in range(4, n_dc):
                k_proj(mo)
                q_proj(mo)
            v_proj(1)
            for hp in range(2, 4):
                attn_block(0, hp)

        # ================= Phase B: rest of attention + Wo + MLP ===========
        with tc.tile_pool(name="p_mlp", bufs=1) as p_mlp, \
             tc.tile_pool(name="p_t2", bufs=2) as p_t2, \
             tc.tile_pool(name="p_wos", bufs=2) as p_wos, \
             tc.tile_pool(name="p_w1", bufs=3) as p_w1, \
             tc.tile_pool(name="p_w2", bufs=2) as p_w2, \
             tc.tile_pool(name="p_y1", bufs=1) as p_y1, \
             tc.tile_pool(name="p_out", bufs=2) as p_out:

            XN2 = p_mlp.tile([P, n_dc, Q], F8)
            y1s = {}

            def wo_block(qq):
                qsl = ts(qq, QQ)
                for mo in range(n_dc):
                    wt = p_wos.tile([P, n_cj, 2, P], F8, tag="wo")
                    nc.sync.dma_start(
                        wt[:],
                        wo_d[:, ts(mo, P)]
                        .rearrange("(c j p) m -> p c j m", j=2, p=P))
                    ps = ps_sh.tile([P, QQ], F32, tag="sh")
                    for cj in range(n_cj):
                        nc.tensor.matmul(
                            ps[:], wt[:, cj, :, :],
                            CT8[:, 2 * cj:2 * cj + 2, qsl],
                            start=(cj == 0), stop=(cj == n_cj - 1),
                            perf_mode=DR)
                    nc.vector.affine_then_add(XQ[:, mo, qsl], ps[:],
                                              XQ[:, mo, qsl],
                                              scale=c_wo,
                                              bias=bo_t[:, mo:mo + 1])

            def ln2_block(qq):
                qsl = ts(qq, QQ)
                st2 = ps_sc.tile([P, 2, QQ], F32, tag="ps_s", name="ps_s")
                for dc in range(n_dc):
                    nc.tensor.matmul(st2[:, 0, :], ones_f[:], XQ[:, dc, qsl],
                                     start=(dc == 0), stop=(dc == n_dc - 1))
                    sq = p_t2.tile([P, QQ], BF16, tag="sq2")
                    nc.gpsimd.tensor_mul(sq[:], XQ[:, dc, qsl],
                                         XQ[:, dc, qsl])
                    nc.tensor.matmul(st2[:, 1, :], ones_f[:], sq[:],
                                     start=(dc == 0), stop=(dc == n_dc - 1))
                mbc = p_t2.tile([P, QQ], F32, tag="mbc2")
                nc.vector.tensor_scalar_mul(mbc[:], st2[:, 0, :], inv_d)
                var = p_t2.tile([P, QQ], F32, tag="var2")
                nc.vector.tensor_scalar(var[:], st2[:, 1, :], inv_d, EPS,
                                        op0=ALU.mult, op1=ALU.add)
                m2 = p_t2.tile([P, QQ], F32, tag="tn2")
                nc.vector.tensor_mul(m2[:], mbc[:], mbc[:])
                nc.vector.tensor_sub(var[:], var[:], m2[:])
                # rstd = rsqrt(var) on DVE only: seed from 1/var + Newton
                r = p_t2.tile([P, QQ], F32, tag="rstd2")
                nc.vector.reciprocal_approx_fast(r[:], var[:])
                nc.vector.tensor_scalar(r[:], r[:], 0.72, 0.35,
                                        op0=ALU.mult, op1=ALU.add)
                for _ in range(3):
                    t1 = p_t2.tile([P, QQ], F32, tag="tn2")
                    nc.vector.tensor_mul(t1[:], r[:], r[:])
                    nc.vector.tensor_mul(t1[:], t1[:], var[:])
                    nc.vector.tensor_scalar(t1[:], t1[:], -0.5, 1.5,
                                            op0=ALU.mult, op1=ALU.add)
                    nc.vector.tensor_mul(r[:], r[:], t1[:])
                for dc in range(n_dc):
                    t0 = p_t2.tile([P, QQ], F32, tag="tn2")
                    nc.vector.tensor_sub(t0[:], XQ[:, dc, qsl], mbc[:])
                    nc.gpsimd.tensor_mul(XN2[:, dc, qsl], t0[:], r[:])

            def fc1_block(qq, mo0, mo1):
                qsl = ts(qq, QQ)
                if qq not in y1s:
                    y1s[qq] = p_y1.tile([P, n_mo, QQ], F8, tag="y1",
                                        name="y1")
                Y1 = y1s[qq]
                for mo in range(mo0, mo1):
                    wt = p_w1.tile([P, n_cj, 2, P], F8, tag="w1")
                    nc.sync.dma_start(
                        wt[:],
                        w1_d[:, ts(mo, P)]
                        .rearrange("(c j p) m -> p c j m", j=2, p=P))
                    ps = ps_sh.tile([P, QQ], F32, tag="sh")
                    for cj in range(n_cj):
                        nc.tensor.matmul(
                            ps[:], wt[:, cj, :, :],
                            XN2[:, 2 * cj:2 * cj + 2, qsl],
                            start=(cj == 0), stop=(cj == n_cj - 1),
                            perf_mode=DR)
                    nc.scalar.activation(Y1[:, mo, :], ps[:], AF.Gelu,
                                         bias=b1_t[:, mo:mo + 1],
                                         scale=inv_s1)

            def fc2_block(qq):
                qsl = ts(qq, QQ)
                Y1 = y1s.pop(qq)
                for mo2 in range(n_dc):
                    wt = p_w2.tile([P, n_m2, 2, P], F8, tag="w2")
                    nc.sync.dma_start(
                        wt[:],
                        w2_d[:, ts(mo2, P)]
                        .rearrange("(c j p) m -> p c j m", j=2, p=P))
                    ps = ps_sh.tile([P, QQ], F32, tag="sh")
                    for cj in range(n_m2):
                        nc.tensor.matmul(
                            ps[:], wt[:, cj, :, :],
                            Y1[:, 2 * cj:2 * cj + 2, :],
                            start=(cj == 0), stop=(cj == n_m2 - 1),
                            perf_mode=DR)
                    ot = p_out.tile([P, QQ], F32, tag="ot")
                    nc.vector.affine_then_add(ot[:], ps[:], XQ[:, mo2, qsl],
                                              scale=inv_s2,
                                              bias=b2_t[:, mo2:mo2 + 1])
                    nc.sync.dma_start(yT_d[ts(mo2, P), qsl], ot[:])

            for hp in range(4, n_hp):
                attn_block(0, hp)
            wo_block(0)
            attn_block(1, 0)
            attn_block(1, 1)
            ln2_block(0)
            fc1_block(0, 0, 16)
            attn_block(1, 2)
            attn_block(1, 3)
            fc1_block(0, 16, 32)
            attn_block(1, 4)
            attn_block(1, 5)
            fc2_block(0)
            attn_block(1, 6)
            attn_block(1, 7)
            wo_block(1)
            ln2_block(1)
            fc1_block(1, 0, n_mo)
            fc2_block(1)
    nc.compile()
    return nc


_NC_CACHE = {}


def _get_nc(T, Q, Dm, Hh, Mlp, n_cores,
            scales=(16.0, 16.0, 16.0, 16.0, 16.0, 16.0, 3.5)):
    key = (T, Q, Dm, Hh, Mlp, n_cores, tuple(scales))
    if key not in _NC_CACHE:
        _NC_CACHE[key] = build_bass(T, Q, Dm, Hh, Mlp, n_cores, scales)
    return _NC_CACHE[key]


def _pow2_scale(absmax, target=128.0):
    a = float(absmax)
    if not np.isfinite(a) or a <= 0:
        return 1.0
    return float(2.0 ** math.floor(math.log2(target / a)))


def prepare(inputs):
    """Host-side prep: LN folding, fp8 quantization, per-core input maps."""
    f = lambda k: np.asarray(inputs[k], np.float32)
    x = f("x")
    Bq, Sq, Dq = x.shape
    Qtok = Sq // 2
    g1, b1ln = f("ln1_g"), f("ln1_b")
    g2, b2ln = f("ln2_g"), f("ln2_b")
    Wq, Wk, Wv, Wo = f("Wq"), f("Wk"), f("Wv"), f("Wo")
    W1, W2 = f("W1"), f("W2")
    bq, bk, bv, bo = f("bq"), f("bk"), f("bv"), f("bo")
    b1, b2 = f("b1"), f("b2")

    # fold LN1 gain/bias into QKV, LN2 gain/bias into W1 (exact)
    Wq_e = g1[:, None] * Wq
    Wk_e = g1[:, None] * Wk
    Wv_e = g1[:, None] * Wv
    bq_e = bq + b1ln @ Wq
    bk_e = bk + b1ln @ Wk
    bv_e = bv + b1ln @ Wv
    W1_e = g2[:, None] * W1
    b1_e = b1 + b2ln @ W1

    s_wq = _pow2_scale(np.abs(Wq_e).max())
    s_wk = _pow2_scale(np.abs(Wk_e).max())
    # V result is stored in fp8 still scaled by s_wv: bound both weight and
    # activation range (sigma of v_j ~ col norm of Wv_e, x is LN'd)
    vcol = np.sqrt((Wv_e ** 2).sum(0))
    vmag = max(float((vcol * 8).max()), float(np.abs(bv_e).max() * 4), 1e-6)
    s_wv = min(_pow2_scale(np.abs(Wv_e).max()),
               _pow2_scale(vmag, target=200.0))
    s_wo = _pow2_scale(np.abs(Wo).max())
    s_w1 = _pow2_scale(np.abs(W1_e).max())
    s_w2 = _pow2_scale(np.abs(W2).max())

    # estimate max attention score for the exp shift C (sampled)
    mu = x.mean(-1, keepdims=True)
    va = x.var(-1, keepdims=True)
    xn_h = (x - mu) / np.sqrt(va + EPS)
    qi = xn_h[:, ::89][:, :16].reshape(-1, Dq)
    ki = xn_h[:, ::13][:, :128].reshape(-1, Dq)
    qp = (qi @ Wq_e + bq_e).reshape(Bq, -1, H, Dq // H)
    kp = (ki @ Wk_e + bk_e).reshape(Bq, -1, H, Dq // H)
    sc = np.einsum("bqhd,bkhd->bhqk", qp, kp) / np.sqrt(Dq // H)
    shift_c = float(sc.max() + 2.0 * sc.std() - math.log(32.0))

    scales = (s_wq, s_wk, s_wv, s_wo, s_w1, s_w2, shift_c)
    nc = _get_nc(Sq, Qtok, Dq, H, MLP, N_CORES, scales)

    shared = {
        "wq8": (Wq_e * s_wq).astype(NP_F8),
        "wk8": (Wk_e * s_wk).astype(NP_F8),
        "wv8": (Wv_e * s_wv).astype(NP_F8),
        "wo8": (Wo * s_wo).astype(NP_F8),
        "w18": (W1_e * s_w1).astype(NP_F8),
        "w28": (W2 * s_w2).astype(NP_F8),
        "bq": (bq_e * s_wq).astype(np.float32),
        "bk": (bk_e * s_wk).astype(np.float32),
        "bv16": (bv_e * s_wv).astype(ml_dtypes.bfloat16),
        "bo": bo.astype(np.float32),
        "b1": b1_e.astype(np.float32),
        "b2": b2.astype(np.float32),
        "ones32": np.ones((P, P), np.float32),
    }
    in_maps = []
    for c in range(N_CORES):
        b = c // 2
        half = c % 2
        xb = x[b]
        xr = np.concatenate(
            [xb[half * Qtok:(half + 1) * Qtok],
             xb[(1 - half) * Qtok:(2 - half) * Qtok]], axis=0)
        m = dict(shared)
        m["xT"] = np.ascontiguousarray(xr.T)
        in_maps.append(m)
    return nc, in_maps, Qtok


def unshard(res, Bq, Sq, Dq, Qtok):
    out = np.empty((Bq, Sq, Dq), np.float32)
    for c in range(N_CORES):
        b = c // 2
        half = c % 2
        out[b, half * Qtok:(half + 1) * Qtok, :] = res.results[c]["yT"].T
    return out


def kernel(**inputs):
    x = np.asarray(inputs["x"], np.float32)
    Bq, Sq, Dq = x.shape
    nc, in_maps, Qtok = prepare(inputs)
    res = run_bass_kernel_spmd(nc, in_maps, core_ids=list(range(N_CORES)))
    return unshard(res, Bq, Sq, Dq, Qtok)


# revision 12
# speedup vs baseline: 1.7241x; 1.0136x over previous
"""Trainium2 Bass kernel for a dense transformer block (LN1 -> MHA -> LN2 -> MLP).

Sharding: 8 cores = (batch b in 0..3) x (sequence half in 0..1), zero
cross-core communication. Each core's input tokens are reordered on the host
so its 1024 query tokens are always tokens 0..1023 of its 2048-token view
(key/value order is irrelevant to attention), letting one SPMD program serve
every core and the query-side LN reuse the full-sequence LN output.

Precision: fp8e4m3 DoubleRow matmuls for QKV/O projections, ctx, and the MLP
(weights pre-scaled by power-of-2 factors on the host; descales fold into
existing bias/scale stages, so they cost nothing). Scores stay bf16.
LayerNorm gain/bias are folded into the following weights on the host
(mathematically exact), so the device LN is a pure (x-mu)*rstd normalize.

Softmax: exp(score - C) with a host-estimated shift C keeping exp outputs in
fp8 range; the denominator is produced by a ones-column appended to V inside
the ctx DoubleRow matmul (out partition 65), so it costs no extra PE time.

Schedule: attention for the first head-pairs is emitted between the two
projection groups so the Act engine's exp stream starts as early as possible;
the MLP of each query half is chunk-interleaved into the next half's
attention so gelus stay clustered (minimizing activation-table reloads) while
PE fills Act-bound stretches. LN2's rstd is computed entirely on DVE
(reciprocal_approx_fast + Newton) to avoid sqrt-table loads mid-stream.
"""

import math
import sys

if '/opt/trn_rl_repo' not in sys.path:
    sys.path.insert(0, '/opt/trn_rl_repo')

import numpy as np
import ml_dtypes

import concourse.tile as tile
import concourse.mybir as mybir
from concourse import bacc
from concourse.bass import ts
from concourse.bass_utils import run_bass_kernel_spmd

P = 128
F32 = mybir.dt.float32
F32R = mybir.dt.float32r
BF16 = mybir.dt.bfloat16
F8 = mybir.dt.float8e4
AF = mybir.ActivationFunctionType
DR = mybir.MatmulPerfMode.DoubleRow
ALU = mybir.AluOpType
EPS = 1e-6

B, S, D, H, MLP = 4, 2048, 1024, 16, 4096
N_CORES = 8
NP_F8 = ml_dtypes.float8_e4m3


def build_bass(T, Q, Dm, Hh, Mlp, n_cores, scales):
    s_wq, s_wk, s_wv, s_wo, s_w1, s_w2, shift_c = scales
    dh = Dm // Hh
    assert dh == 64
    n_dc = Dm // P          # 8 feature chunks
    n_cj = n_dc // 2        # 4 DoubleRow k-pair steps over D
    n_tk = T // P           # 16 token chunks
    TB = 512
    n_tb = T // TB          # 4
    QQ = 512
    n_qq = Q // QQ          # 2
    n_mo = Mlp // P         # 32
    n_m2 = n_mo // 2        # 16 DoubleRow k-pair steps over MLP
    n_hp = Hh // 2          # 8 head pairs
    inv_d = 1.0 / Dm
    exp_scale = 0.125 / (s_wq * s_wk)
    c_wo = 1.0 / (s_wo * s_wv)
    inv_s1 = 1.0 / s_w1
    inv_s2 = 1.0 / s_w2

    nc = bacc.Bacc("TRN2", target_bir_lowering=False, debug=False,
                   enable_asserts=False, num_devices=n_cores)

    def din(name, shape, dt):
        return nc.dram_tensor(name, shape, dt, kind="ExternalInput").ap()

    xT_d = din("xT", (Dm, T), F32)
    wq_d, wk_d = din("wq8", (Dm, Dm), F8), din("wk8", (Dm, Dm), F8)
    wv_d, wo_d = din("wv8", (Dm, Dm), F8), din("wo8", (Dm, Dm), F8)
    w1_d = din("w18", (Dm, Mlp), F8)
    w2_d = din("w28", (Mlp, Dm), F8)
    bq_d, bk_d = din("bq", (Dm,), F32), din("bk", (Dm,), F32)
    bv_d, bo_d = din("bv16", (Dm,), BF16), din("bo", (Dm,), F32)
    b1_d, b2_d = din("b1", (Mlp,), F32), din("b2", (Dm,), F32)
    ones_d = din("ones32", (P, P), F32)
    yT_d = nc.dram_tensor("yT", (Dm, Q), F32, kind="ExternalOutput").ap()

    with tile.TileContext(nc) as tc, \
         tc.tile_pool(name="const", bufs=1) as constp, \
         tc.tile_pool(name="p_res", bufs=1) as p_res, \
         tc.tile_pool(name="p_kv", bufs=1) as p_kv, \
         tc.tile_pool(name="p_exp", bufs=3) as p_exp, \
         tc.tile_pool(name="p_rb", bufs=2) as p_rb, \
         tc.tile_pool(name="p_ct", bufs=1) as p_ct, \
         tc.tile_pool(name="ps_sc", bufs=2, space="PSUM") as ps_sc, \
         tc.tile_pool(name="ps_ctx", bufs=2, space="PSUM") as ps_ctx, \
         tc.tile_pool(name="ps_sh", bufs=2, space="PSUM") as ps_sh:

        ones_fr = constp.tile([P, P], F32R)
        nc.sync.dma_start(ones_fr[:], ones_d[:, :].bitcast(F32R))
        ones_f = constp.tile([P, P], BF16)
        nc.vector.memset(ones_f[:], 1.0)
        eps_t = constp.tile([P, 1], F32)
        nc.vector.memset(eps_t[:], EPS)
        negc_t = constp.tile([P, 1], F32)
        nc.vector.memset(negc_t[:], -shift_c)

        def vec_tile(src, n, nm):
            t = constp.tile([P, n], F32, tag=nm, name=nm)
            nc.sync.dma_start(t[:], src.rearrange("(c p) -> p c", p=P))
            return t

        bq_t, bk_t = vec_tile(bq_d, n_dc, "bq"), vec_tile(bk_d, n_dc, "bk")
        bo_t, b2_t = vec_tile(bo_d, n_dc, "bo"), vec_tile(b2_d, n_dc, "b2")
        b1_t = vec_tile(b1_d, n_mo, "b1")

        XQ = p_res.tile([P, n_dc, Q], BF16)       # residual stream (bf16)
        KT = p_kv.tile([P, n_dc, T], BF16)
        QT = p_kv.tile([P, n_dc, Q], BF16)
        VT = p_kv.tile([P, n_tk, Hh, 66], F8)
        nc.gpsimd.memset(VT[:, :, :, 64:66], 1.0)
        CT8 = p_ct.tile([P, n_dc, Q], F8)

        def attn_block(qq, hp):
            qsl = ts(qq, QQ)
            exps = [p_exp.tile([P, n_tk, QQ], F8, tag="expT", name="expT")
                    for _ in range(2)]
            for kcp in range(n_tk // 2):
                for hi in range(2):
                    r0 = hi * 64
                    pss = ps_sc.tile([P, 2, QQ], F32, tag="ps_s", name="ps_s")
                    for j in range(2):
                        nc.tensor.matmul(
                            pss[:, j, :],
                            KT[r0:r0 + 64, hp, ts(2 * kcp + j, P)],
                            QT[r0:r0 + 64, hp, qsl],
                            start=True, stop=True)
                    nc.scalar.activation(
                        exps[hi][:, 2 * kcp:2 * kcp + 2, :],
                        pss[:, :, :], AF.Exp,
                        scale=exp_scale, bias=negc_t[:, 0:1])
            pcs = [ps_ctx.tile([65, QQ], F32, tag="ps_c", name="ps_c")
                   for _ in range(2)]
            for kcp in range(n_tk // 2):
                for hi in range(2):
                    h = 2 * hp + hi
                    nc.tensor.matmul(
                        pcs[hi][:, :],
                        VT[:, 2 * kcp:2 * kcp + 2, h, 0:65],
                        exps[hi][:, 2 * kcp:2 * kcp + 2, :],
                        start=(kcp == 0), stop=(kcp == n_tk // 2 - 1),
                        perf_mode=DR)
            for hi in range(2):
                r0 = hi * 64
                rb1 = p_rb.tile([1, QQ], F32, tag="rb1")
                nc.vector.reciprocal_approx_fast(rb1[:], pcs[hi][64:65, :])
                rbb = p_rb.tile([64, QQ], F32, tag="rbb")
                nc.gpsimd.partition_broadcast(rbb[:], rb1[:])
                nc.vector.tensor_mul(CT8[r0:r0 + 64, hp, qsl],
                                     pcs[hi][0:64, :], rbb[:])

        # ================= Phase A: LN1 + projections (+early attn) ========
        with tc.tile_pool(name="p_xn8", bufs=1) as p_xn8, \
             tc.tile_pool(name="p_x", bufs=2) as p_x, \
             tc.tile_pool(name="p_t", bufs=2) as p_t, \
             tc.tile_pool(name="p_w", bufs=3) as p_w, \
             tc.tile_pool(name="p_wv", bufs=2) as p_wv:

            XN8 = p_xn8.tile([P, n_dc, T], F8)
            bv_row = p_xn8.tile([1, Dm], BF16)
            nc.sync.dma_start(bv_row[:, :], bv_d[None, :])
            bv_bc = p_xn8.tile([P, Dm], BF16)
            nc.gpsimd.partition_broadcast(bv_bc[:], bv_row[:])

            for tb in range(n_tb):
                tsl = ts(tb, TB)
                xc = p_x.tile([P, n_dc, TB], F32R, tag="xc")
                for dc in range(n_dc):
                    nc.sync.dma_start(xc[:, dc, :],
                                      xT_d[ts(dc, P), tsl].bitcast(F32R))
                st = ps_sc.tile([P, 2, TB], F32, tag="ps_s", name="ps_s")
                for dc in range(n_dc):
                    nc.tensor.matmul(st[:, 0, :], ones_fr[:], xc[:, dc, :],
                                     start=(dc == 0), stop=(dc == n_dc - 1))
                    xsq = p_t.tile([P, TB], BF16, tag="xsq")
                    nc.scalar.activation(xsq[:], xc[:, dc, :].bitcast(F32),
                                         AF.Square)
                    nc.tensor.matmul(st[:, 1, :], ones_f[:], xsq[:],
                                     start=(dc == 0), stop=(dc == n_dc - 1))
                    if tb < Q // TB:
                        nc.vector.tensor_copy(XQ[:, dc, tsl],
                                              xc[:, dc, :].bitcast(F32))
                mbc = p_t.tile([P, TB], F32, tag="mbc")
                nc.vector.tensor_scalar_mul(mbc[:], st[:, 0, :], inv_d)
                var = p_t.tile([P, TB], F32, tag="var")
                nc.vector.tensor_scalar(var[:], st[:, 1, :], inv_d, EPS,
                                        op0=ALU.mult, op1=ALU.add)
                m2 = p_t.tile([P, TB], F32, tag="tn")
                nc.vector.tensor_mul(m2[:], mbc[:], mbc[:])
                nc.vector.tensor_sub(var[:], var[:], m2[:])
                std = p_t.tile([P, TB], F32, tag="stdt")
                nc.scalar.activation(std[:], var[:], AF.Sqrt)
                rstd = p_t.tile([P, TB], F32, tag="rstd")
                nc.vector.reciprocal(rstd[:], std[:])
                for dc in range(n_dc):
                    t0 = p_t.tile([P, TB], F32, tag="tn")
                    nc.vector.tensor_sub(t0[:], xc[:, dc, :].bitcast(F32),
                                         mbc[:])
                    nc.gpsimd.tensor_mul(XN8[:, dc, tsl], t0[:], rstd[:])

            def q_proj(mo):
                wt = p_w.tile([P, n_cj, 2, P], F8, tag="wq")
                nc.sync.dma_start(
                    wt[:],
                    wq_d[:, ts(mo, P)]
                    .rearrange("(c j p) m -> p c j m", j=2, p=P))
                for qb in range(n_qq):
                    ps = ps_sh.tile([P, QQ], F32, tag="sh")
                    for cj in range(n_cj):
                        nc.tensor.matmul(
                            ps[:], wt[:, cj, :, :],
                            XN8[:, 2 * cj:2 * cj + 2, ts(qb, QQ)],
                            start=(cj == 0), stop=(cj == n_cj - 1),
                            perf_mode=DR)
                    nc.vector.tensor_scalar_add(QT[:, mo, ts(qb, QQ)], ps[:],
                                                bq_t[:, mo:mo + 1])

            def k_proj(mo):
                wt = p_w.tile([P, n_cj, 2, P], F8, tag="wq")
                nc.sync.dma_start(
                    wt[:],
                    wk_d[:, ts(mo, P)]
                    .rearrange("(c j p) m -> p c j m", j=2, p=P))
                for tb in range(n_tb):
                    ps = ps_sh.tile([P, TB], F32, tag="sh")
                    for cj in range(n_cj):
                        nc.tensor.matmul(
                            ps[:], wt[:, cj, :, :],
                            XN8[:, 2 * cj:2 * cj + 2, ts(tb, TB)],
                            start=(cj == 0), stop=(cj == n_cj - 1),
                            perf_mode=DR)
                    nc.vector.tensor_scalar_add(KT[:, mo, ts(tb, TB)], ps[:],
                                                bk_t[:, mo:mo + 1])

            def v_proj(no):
                NO = 512
                wt = p_wv.tile([P, n_cj, 2, NO], F8, tag="wv")
                nc.sync.dma_start(
                    wt[:],
                    wv_d[:, ts(no, NO)]
                    .rearrange("(c j p) m -> p c j m", j=2, p=P))
                for to in range(n_tk):
                    ps = ps_sh.tile([P, NO], F32, tag="sh")
                    for cj in range(n_cj):
                        nc.tensor.matmul(
                            ps[:], XN8[:, 2 * cj:2 * cj + 2, ts(to, P)],
                            wt[:, cj, :, :],
                            start=(cj == 0), stop=(cj == n_cj - 1),
                            perf_mode=DR)
                    nc.vector.tensor_add(VT[:, to, 8 * no:8 * no + 8, 0:64],
                                         ps[:], bv_bc[:, ts(no, NO)])

            for mo in range(4):
                k_proj(mo)
                q_proj(mo)
            v_proj(0)
            for hp in range(2):
                attn_block(0, hp)
            for mo in range(4, n_dc):
                k_proj(mo)
                q_proj(mo)
            v_proj(1)
            for hp in range(2, 4):
                attn_block(0, hp)

        # ================= Phase B: rest of attention + Wo + MLP ===========
        with tc.tile_pool(name="p_mlp", bufs=1) as p_mlp, \
             tc.tile_pool(name="p_t2", bufs=2) as p_t2, \
             tc.tile_pool(name="p_wos", bufs=2) as p_wos, \
             tc.tile_pool(name="p_w1", bufs=3) as p_w1, \
             tc.tile_pool(name="p_w2", bufs=2) as p_w2, \
             tc.tile_pool(name="p_y1", bufs=1) as p_y1, \
             tc.tile_pool(name="p_out", bufs=2) as p_out:

            XN2 = p_mlp.tile([P, n_dc, Q], F8)
            y1s = {}

            def wo_block(qq):
                qsl = ts(qq, QQ)
                for mo in range(n_dc):
                    wt = p_wos.tile([P, n_cj, 2, P], F8, tag="wo")
                    nc.sync.dma_start(
                        wt[:],
                        wo_d[:, ts(mo, P)]
                        .rearrange("(c j p) m -> p c j m", j=2, p=P))
                    ps = ps_sh.tile([P, QQ], F32, tag="sh")
                    for cj in range(n_cj):
                        nc.tensor.matmul(
                            ps[:], wt[:, cj, :, :],
                            CT8[:, 2 * cj:2 * cj + 2, qsl],
                            start=(cj == 0), stop=(cj == n_cj - 1),
                            perf_mode=DR)
                    nc.vector.affine_then_add(XQ[:, mo, qsl], ps[:],
                                              XQ[:, mo, qsl],
                                              scale=c_wo,
                                              bias=bo_t[:, mo:mo + 1])

            def ln2_block(qq):
                qsl = ts(qq, QQ)
                st2 = ps_sc.tile([P, 2, QQ], F32, tag="ps_s", name="ps_s")
                for dc in range(n_dc):
                    nc.tensor.matmul(st2[:, 0, :], ones_f[:], XQ[:, dc, qsl],
                                     start=(dc == 0), stop=(dc == n_dc - 1))
                    sq = p_t2.tile([P, QQ], BF16, tag="sq2")
                    nc.gpsimd.tensor_mul(sq[:], XQ[:, dc, qsl],
                                         XQ[:, dc, qsl])
                    nc.tensor.matmul(st2[:, 1, :], ones_f[:], sq[:],
                                     start=(dc == 0), stop=(dc == n_dc - 1))
                mbc = p_t2.tile([P, QQ], F32, tag="mbc2")
                nc.vector.tensor_scalar_mul(mbc[:], st2[:, 0, :], inv_d)
                var = p_t2.tile([P, QQ], F32, tag="var2")
                nc.vector.tensor_scalar(var[:], st2[:, 1, :], inv_d, EPS,
                                        op0=ALU.mult, op1=ALU.add)
                m2 = p_t2.tile([P, QQ], F32, tag="tn2")
                nc.vector.tensor_mul(m2[:], mbc[:], mbc[:])
                nc.vector.tensor_sub(var[:], var[:], m2[:])
                # rstd = rsqrt(var) on DVE only: seed from 1/var + Newton
                r = p_t2.tile([P, QQ], F32, tag="rstd2")
                nc.vector.reciprocal_approx_fast(r[:], var[:])
                nc.vector.tensor_scalar(r[:], r[:], 0.72, 0.35,
                                        op0=ALU.mult, op1=ALU.add)
                for _ in range(3):
                    t1 = p_t2.tile([P, QQ], F32, tag="tn2")
                    nc.vector.tensor_mul(t1[:], r[:], r[:])
                    nc.vector.tensor_mul(t1[:], t1[:], var[:])
                    nc.vector.tensor_scalar(t1[:], t1[:], -0.5, 1.5,
                                            op0=ALU.mult, op1=ALU.add)
                    nc.vector.tensor_mul(r[:], r[:], t1[:])
                for dc in range(n_dc):
                    t0 = p_t2.tile([P, QQ], F32, tag="tn2")
                    nc.vector.tensor_sub(t0[:], XQ[:, dc, qsl], mbc[:])
                    nc.gpsimd.tensor_mul(XN2[:, dc, qsl], t0[:], r[:])

            def fc1_block(qq, mo0, mo1):
                qsl = ts(qq, QQ)
                if qq not in y1s:
                    y1s[qq] = p_y1.tile([P, n_mo, QQ], F8, tag="y1",
                                        name="y1")
                Y1 = y1s[qq]
                for mo in range(mo0, mo1):
                    wt = p_w1.tile([P, n_cj, 2, P], F8, tag="w1")
                    nc.sync.dma_start(
                        wt[:],
                        w1_d[:, ts(mo, P)]
                        .rearrange("(c j p) m -> p c j m", j=2, p=P))
                    ps = ps_sh.tile([P, QQ], F32, tag="sh")
                    for cj in range(n_cj):
                        nc.tensor.matmul(
                            ps[:], wt[:, cj, :, :],
                            XN2[:, 2 * cj:2 * cj + 2, qsl],
                            start=(cj == 0), stop=(cj == n_cj - 1),
                            perf_mode=DR)
                    nc.scalar.activation(Y1[:, mo, :], ps[:], AF.Gelu,
                                         bias=b1_t[:, mo:mo + 1],
                                         scale=inv_s1)

            def fc2_block(qq):
                qsl = ts(qq, QQ)
                Y1 = y1s.pop(qq)
                for mo2 in range(n_dc):
                    wt = p_w2.tile([P, n_m2, 2, P], F8, tag="w2")
                    nc.sync.dma_start(
                        wt[:],
                        w2_d[:, ts(mo2, P)]
                        .rearrange("(c j p) m -> p c j m", j=2, p=P))
                    ps = ps_sh.tile([P, QQ], F32, tag="sh")
                    for cj in range(n_m2):
                        nc.tensor.matmul(
                            ps[:], wt[:, cj, :, :],
                            Y1[:, 2 * cj:2 * cj + 2, :],
                            start=(cj == 0), stop=(cj == n_m2 - 1),
                            perf_mode=DR)
                    ot = p_out.tile([P, QQ], F32, tag="ot")
                    nc.vector.affine_then_add(ot[:], ps[:], XQ[:, mo2, qsl],
                                              scale=inv_s2,
                                              bias=b2_t[:, mo2:mo2 + 1])
                    nc.sync.dma_start(yT_d[ts(mo2, P), qsl], ot[:])

            for hp in range(4, n_hp):
                attn_block(0, hp)
            wo_block(0)
            attn_block(1, 0)
            attn_block(1, 1)
            ln2_block(0)
            for hp in range(2, n_hp):
                attn_block(1, hp)
            wo_block(1)
            fc1_block(0, 0, n_mo)
            ln2_block(1)
            fc2_block(0)
            fc1_block(1, 0, n_mo)
            fc2_block(1)
    nc.compile()
    return nc


_NC_CACHE = {}


def _get_nc(T, Q, Dm, Hh, Mlp, n_cores,
            scales=(16.0, 16.0, 16.0, 16.0, 16.0, 16.0, 3.5)):
    key = (T, Q, Dm, Hh, Mlp, n_cores, tuple(scales))
    if key not in _NC_CACHE:
        _NC_CACHE[key] = build_bass(T, Q, Dm, Hh, Mlp, n_cores, scales)
    return _NC_CACHE[key]


def _pow2_scale(absmax, target=128.0):
    a = float(absmax)
    if not np.isfinite(a) or a <= 0:
        return 1.0
    return float(2.0 ** math.floor(math.log2(target / a)))


def prepare(inputs):
    """Host-side prep: LN folding, fp8 quantization, per-core input maps."""
    f = lambda k: np.asarray(inputs[k], np.float32)
    x = f("x")
    Bq, Sq, Dq = x.shape
    Qtok = Sq // 2
    g1, b1ln = f("ln1_g"), f("ln1_b")
    g2, b2ln = f("ln2_g"), f("ln2_b")
    Wq, Wk, Wv, Wo = f("Wq"), f("Wk"), f("Wv"), f("Wo")
    W1, W2 = f("W1"), f("W2")
    bq, bk, bv, bo = f("bq"), f("bk"), f("bv"), f("bo")
    b1, b2 = f("b1"), f("b2")

    # fold LN1 gain/bias into QKV, LN2 gain/bias into W1 (exact)
    Wq_e = g1[:, None] * Wq
    Wk_e = g1[:, None] * Wk
    Wv_e = g1[:, None] * Wv
    bq_e = bq + b1ln @ Wq
    bk_e = bk + b1ln @ Wk
    bv_e = bv + b1ln @ Wv
    W1_e = g2[:, None] * W1
    b1_e = b1 + b2ln @ W1

    s_wq = _pow2_scale(np.abs(Wq_e).max())
    s_wk = _pow2_scale(np.abs(Wk_e).max())
    # V result is stored in fp8 still scaled by s_wv: bound both weight and
    # activation range (sigma of v_j ~ col norm of Wv_e, x is LN'd)
    vcol = np.sqrt((Wv_e ** 2).sum(0))
    vmag = max(float((vcol * 8).max()), float(np.abs(bv_e).max() * 4), 1e-6)
    s_wv = min(_pow2_scale(np.abs(Wv_e).max()),
               _pow2_scale(vmag, target=200.0))
    s_wo = _pow2_scale(np.abs(Wo).max())
    s_w1 = _pow2_scale(np.abs(W1_e).max())
    s_w2 = _pow2_scale(np.abs(W2).max())

    # estimate max attention score for the exp shift C (sampled)
    mu = x.mean(-1, keepdims=True)
    va = x.var(-1, keepdims=True)
    xn_h = (x - mu) / np.sqrt(va + EPS)
    qi = xn_h[:, ::89][:, :16].reshape(-1, Dq)
    ki = xn_h[:, ::13][:, :128].reshape(-1, Dq)
    qp = (qi @ Wq_e + bq_e).reshape(Bq, -1, H, Dq // H)
    kp = (ki @ Wk_e + bk_e).reshape(Bq, -1, H, Dq // H)
    sc = np.einsum("bqhd,bkhd->bhqk", qp, kp) / np.sqrt(Dq // H)
    shift_c = float(sc.max() + 2.0 * sc.std() - math.log(32.0))

    scales = (s_wq, s_wk, s_wv, s_wo, s_w1, s_w2, shift_c)
    nc = _get_nc(Sq, Qtok, Dq, H, MLP, N_CORES, scales)

    shared = {
        "wq8": (Wq_e * s_wq).astype(NP_F8),
        "wk8": (Wk_e * s_wk).astype(NP_F8),
        "wv8": (Wv_e * s_wv).astype(NP_F8),
        "wo8": (Wo * s_wo).astype(NP_F8),
        "w18": (W1_e * s_w1).astype(NP_F8),
        "w28": (W2 * s_w2).astype(NP_F8),
        "bq": (bq_e * s_wq).astype(np.float32),
        "bk": (bk_e * s_wk).astype(np.float32),
        "bv16": (bv_e * s_wv).astype(ml_dtypes.bfloat16),
        "bo": bo.astype(np.float32),
        "b1": b1_e.astype(np.float32),
        "b2": b2.astype(np.float32),
        "ones32": np.ones((P, P), np.float32),
    }
    in_maps = []
    for c in range(N_CORES):
        b = c // 2
        half = c % 2
        xb = x[b]
        xr = np.concatenate(
            [xb[half * Qtok:(half + 1) * Qtok],
             xb[(1 - half) * Qtok:(2 - half) * Qtok]], axis=0)
        m = dict(shared)
        m["xT"] = np.ascontiguousarray(xr.T)
        in_maps.append(m)
    return nc, in_maps, Qtok


def unshard(res, Bq, Sq, Dq, Qtok):
    out = np.empty((Bq, Sq, Dq), np.float32)
    for c in range(N_CORES):
        b = c // 2
        half = c % 2
        out[b, half * Qtok:(half + 1) * Qtok, :] = res.results[c]["yT"].T
    return out


def kernel(**inputs):
    x = np.asarray(inputs["x"], np.float32)
    Bq, Sq, Dq = x.shape
    nc, in_maps, Qtok = prepare(inputs)
    res = run_bass_kernel_spmd(nc, in_maps, core_ids=list(range(N_CORES)))
    return unshard(res, Bq, Sq, Dq, Qtok)


# revision 14
# speedup vs baseline: 1.9268x; 1.1176x over previous
"""Trainium2 Bass kernel for a dense transformer block (LN1 -> MHA -> LN2 -> MLP).

Sharding: 8 cores = (batch b in 0..3) x (sequence half in 0..1), zero
cross-core communication. Each core's input tokens are reordered on the host
so its 1024 query tokens are always tokens 0..1023 of its 2048-token view
(key/value order is irrelevant to attention), letting one SPMD program serve
every core and the query-side LN reuse the full-sequence LN output.

Precision: fp8e4m3 DoubleRow matmuls for QKV/O projections, ctx, and the MLP
(weights pre-scaled by power-of-2 factors on the host; descales fold into
existing bias/scale stages, so they cost nothing). Scores stay bf16.
LayerNorm gain/bias are folded into the following weights on the host
(mathematically exact), so the device LN is a pure (x-mu)*rstd normalize.

Softmax: exp(score - C) with a host-estimated shift C keeping exp outputs in
fp8 range; the denominator is produced by a ones-column appended to V inside
the ctx DoubleRow matmul (out partition 65), so it costs no extra PE time.

Schedule: attention for the first head-pairs is emitted between the two
projection groups so the Act engine's exp stream starts as early as possible;
the MLP of each query half is chunk-interleaved into the next half's
attention so gelus stay clustered (minimizing activation-table reloads) while
PE fills Act-bound stretches. LN2's rstd is computed entirely on DVE
(reciprocal_approx_fast + Newton) to avoid sqrt-table loads mid-stream.
"""

import math
import sys

if '/opt/trn_rl_repo' not in sys.path:
    sys.path.insert(0, '/opt/trn_rl_repo')

import numpy as np
import ml_dtypes

import concourse.tile as tile
import concourse.mybir as mybir
from concourse import bacc
from concourse.bass import ts
from concourse.bass_utils import run_bass_kernel_spmd

P = 128
F32 = mybir.dt.float32
F32R = mybir.dt.float32r
BF16 = mybir.dt.bfloat16
F8 = mybir.dt.float8e4
AF = mybir.ActivationFunctionType
DR = mybir.MatmulPerfMode.DoubleRow
ALU = mybir.AluOpType
EPS = 1e-6

B, S, D, H, MLP = 4, 2048, 1024, 16, 4096
N_CORES = 8
NP_F8 = ml_dtypes.float8_e4m3


def build_bass(T, Q, Dm, Hh, Mlp, n_cores, scales):
    s_wq, s_wk, s_wv, s_wo, s_w1, s_w2, shift_c = scales
    dh = Dm // Hh
    assert dh == 64
    n_dc = Dm // P          # 8 feature chunks
    n_cj = n_dc // 2        # 4 DoubleRow k-pair steps over D
    n_tk = T // P           # 16 token chunks
    TB = 512
    n_tb = T // TB          # 4
    QQ = 512
    n_qq = Q // QQ          # 2
    n_mo = Mlp // P         # 32
    n_m2 = n_mo // 2        # 16 DoubleRow k-pair steps over MLP
    n_hp = Hh // 2          # 8 head pairs
    inv_d = 1.0 / Dm
    exp_scale = 0.125 / (s_wq * s_wk)
    c_wo = 1.0 / (s_wo * s_wv)
    inv_s1 = 1.0 / s_w1
    inv_s2 = 1.0 / s_w2

    nc = bacc.Bacc("TRN2", target_bir_lowering=False, debug=False,
                   enable_asserts=False, num_devices=n_cores)

    def din(name, shape, dt):
        return nc.dram_tensor(name, shape, dt, kind="ExternalInput").ap()

    xT_d = din("xT", (Dm, T), F32)
    wq_d, wk_d = din("wq8", (Dm, Dm), F8), din("wk8", (Dm, Dm), F8)
    wv_d, wo_d = din("wv8", (Dm, Dm), F8), din("wo8", (Dm, Dm), F8)
    w1_d = din("w18", (Dm, Mlp), F8)
    w2_d = din("w28", (Mlp, Dm), F8)
    bq_d, bk_d = din("bq", (Dm,), F32), din("bk", (Dm,), F32)
    bv_d, bo_d = din("bv16", (Dm,), BF16), din("bo", (Dm,), F32)
    b1_d, b2_d = din("b1", (Mlp,), F32), din("b2", (Dm,), F32)
    ones_d = din("ones32", (P, P), F32)
    yT_d = nc.dram_tensor("yT", (Dm, Q), F32, kind="ExternalOutput").ap()

    with tile.TileContext(nc) as tc, \
         tc.tile_pool(name="const", bufs=1) as constp, \
         tc.tile_pool(name="p_res", bufs=1) as p_res, \
         tc.tile_pool(name="p_kv", bufs=1) as p_kv, \
         tc.tile_pool(name="p_exp", bufs=3) as p_exp, \
         tc.tile_pool(name="p_rb", bufs=2) as p_rb, \
         tc.tile_pool(name="p_ct", bufs=1) as p_ct, \
         tc.tile_pool(name="ps_sc", bufs=2, space="PSUM") as ps_sc, \
         tc.tile_pool(name="ps_ctx", bufs=2, space="PSUM") as ps_ctx, \
         tc.tile_pool(name="ps_sh", bufs=2, space="PSUM") as ps_sh:

        ones_fr = constp.tile([P, P], F32R)
        nc.sync.dma_start(ones_fr[:], ones_d[:, :].bitcast(F32R))
        ones_f = constp.tile([P, P], BF16)
        nc.vector.memset(ones_f[:], 1.0)
        eps_t = constp.tile([P, 1], F32)
        nc.vector.memset(eps_t[:], EPS)
        negc_t = constp.tile([P, 1], F32)
        nc.vector.memset(negc_t[:], -shift_c)

        def vec_tile(src, n, nm):
            t = constp.tile([P, n], F32, tag=nm, name=nm)
            nc.sync.dma_start(t[:], src.rearrange("(c p) -> p c", p=P))
            return t

        bq_t, bk_t = vec_tile(bq_d, n_dc, "bq"), vec_tile(bk_d, n_dc, "bk")
        bo_t, b2_t = vec_tile(bo_d, n_dc, "bo"), vec_tile(b2_d, n_dc, "b2")
        b1_t = vec_tile(b1_d, n_mo, "b1")

        XQ = p_res.tile([P, n_dc, Q], BF16)       # residual stream (bf16)
        KT = p_kv.tile([P, n_dc, T], BF16)
        QT = p_kv.tile([P, n_dc, Q], BF16)
        VT = p_kv.tile([P, n_tk, Hh, 66], F8)
        nc.gpsimd.memset(VT[:, :, :, 64:66], 1.0)
        CT8 = p_ct.tile([P, n_dc, Q], F8)

        def attn_block(qq, hp):
            qsl = ts(qq, QQ)
            exps = [p_exp.tile([P, n_tk, QQ], F8, tag="expT", name="expT")
                    for _ in range(2)]
            for kcp in range(n_tk // 2):
                for hi in range(2):
                    r0 = hi * 64
                    pss = ps_sc.tile([P, 2, QQ], F32, tag="ps_s", name="ps_s")
                    for j in range(2):
                        nc.tensor.matmul(
                            pss[:, j, :],
                            KT[r0:r0 + 64, hp, ts(2 * kcp + j, P)],
                            QT[r0:r0 + 64, hp, qsl],
                            start=True, stop=True)
                    nc.scalar.activation(
                        exps[hi][:, 2 * kcp:2 * kcp + 2, :],
                        pss[:, :, :], AF.Exp,
                        scale=exp_scale, bias=negc_t[:, 0:1])
            pcs = [ps_ctx.tile([65, QQ], F32, tag="ps_c", name="ps_c")
                   for _ in range(2)]
            for kcp in range(n_tk // 2):
                for hi in range(2):
                    h = 2 * hp + hi
                    nc.tensor.matmul(
                        pcs[hi][:, :],
                        VT[:, 2 * kcp:2 * kcp + 2, h, 0:65],
                        exps[hi][:, 2 * kcp:2 * kcp + 2, :],
                        start=(kcp == 0), stop=(kcp == n_tk // 2 - 1),
                        perf_mode=DR)
            for hi in range(2):
                r0 = hi * 64
                rb1 = p_rb.tile([1, QQ], F32, tag="rb1")
                nc.vector.reciprocal_approx_fast(rb1[:], pcs[hi][64:65, :])
                rbb = p_rb.tile([64, QQ], F32, tag="rbb")
                nc.gpsimd.partition_broadcast(rbb[:], rb1[:])
                nc.vector.tensor_mul(CT8[r0:r0 + 64, hp, qsl],
                                     pcs[hi][0:64, :], rbb[:])

        # ================= Phase A: LN1 + projections (+early attn) ========
        with tc.tile_pool(name="p_xn8", bufs=1) as p_xn8, \
             tc.tile_pool(name="p_x", bufs=2) as p_x, \
             tc.tile_pool(name="p_t", bufs=2) as p_t, \
             tc.tile_pool(name="p_w", bufs=3) as p_w, \
             tc.tile_pool(name="p_wv", bufs=2) as p_wv:

            XN8 = p_xn8.tile([P, n_dc, T], F8)
            bv_row = p_xn8.tile([1, Dm], BF16)
            nc.sync.dma_start(bv_row[:, :], bv_d[None, :])
            bv_bc = p_xn8.tile([P, Dm], BF16)
            nc.gpsimd.partition_broadcast(bv_bc[:], bv_row[:])

            for tb in range(n_tb):
                tsl = ts(tb, TB)
                xc = p_x.tile([P, n_dc, TB], F32R, tag="xc")
                for dc in range(n_dc):
                    nc.sync.dma_start(xc[:, dc, :],
                                      xT_d[ts(dc, P), tsl].bitcast(F32R))
                st = ps_sc.tile([P, 2, TB], F32, tag="ps_s", name="ps_s")
                for dc in range(n_dc):
                    nc.tensor.matmul(st[:, 0, :], ones_fr[:], xc[:, dc, :],
                                     start=(dc == 0), stop=(dc == n_dc - 1))
                    xsq = p_t.tile([P, TB], BF16, tag="xsq")
                    nc.scalar.activation(xsq[:], xc[:, dc, :].bitcast(F32),
                                         AF.Square)
                    nc.tensor.matmul(st[:, 1, :], ones_f[:], xsq[:],
                                     start=(dc == 0), stop=(dc == n_dc - 1))
                    if tb < Q // TB:
                        nc.vector.tensor_copy(XQ[:, dc, tsl],
                                              xc[:, dc, :].bitcast(F32))
                mbc = p_t.tile([P, TB], F32, tag="mbc")
                nc.vector.tensor_scalar_mul(mbc[:], st[:, 0, :], inv_d)
                var = p_t.tile([P, TB], F32, tag="var")
                nc.vector.tensor_scalar(var[:], st[:, 1, :], inv_d, EPS,
                                        op0=ALU.mult, op1=ALU.add)
                m2 = p_t.tile([P, TB], F32, tag="tn")
                nc.vector.tensor_mul(m2[:], mbc[:], mbc[:])
                nc.vector.tensor_sub(var[:], var[:], m2[:])
                std = p_t.tile([P, TB], F32, tag="stdt")
                nc.scalar.activation(std[:], var[:], AF.Sqrt)
                rstd = p_t.tile([P, TB], F32, tag="rstd")
                nc.vector.reciprocal(rstd[:], std[:])
                for dc in range(n_dc):
                    t0 = p_t.tile([P, TB], F32, tag="tn")
                    nc.vector.tensor_sub(t0[:], xc[:, dc, :].bitcast(F32),
                                         mbc[:])
                    nc.gpsimd.tensor_mul(XN8[:, dc, tsl], t0[:], rstd[:])

            def q_proj(mo):
                wt = p_w.tile([P, n_cj, 2, P], F8, tag="wq")
                nc.sync.dma_start(
                    wt[:],
                    wq_d[:, ts(mo, P)]
                    .rearrange("(c j p) m -> p c j m", j=2, p=P))
                for qb in range(n_qq):
                    ps = ps_sh.tile([P, QQ], F32, tag="sh")
                    for cj in range(n_cj):
                        nc.tensor.matmul(
                            ps[:], wt[:, cj, :, :],
                            XN8[:, 2 * cj:2 * cj + 2, ts(qb, QQ)],
                            start=(cj == 0), stop=(cj == n_cj - 1),
                            perf_mode=DR)
                    nc.vector.tensor_scalar_add(QT[:, mo, ts(qb, QQ)], ps[:],
                                                bq_t[:, mo:mo + 1])

            def k_proj(mo):
                wt = p_w.tile([P, n_cj, 2, P], F8, tag="wq")
                nc.sync.dma_start(
                    wt[:],
                    wk_d[:, ts(mo, P)]
                    .rearrange("(c j p) m -> p c j m", j=2, p=P))
                for tb in range(n_tb):
                    ps = ps_sh.tile([P, TB], F32, tag="sh")
                    for cj in range(n_cj):
                        nc.tensor.matmul(
                            ps[:], wt[:, cj, :, :],
                            XN8[:, 2 * cj:2 * cj + 2, ts(tb, TB)],
                            start=(cj == 0), stop=(cj == n_cj - 1),
                            perf_mode=DR)
                    nc.vector.tensor_scalar_add(KT[:, mo, ts(tb, TB)], ps[:],
                                                bk_t[:, mo:mo + 1])

            def v_proj(no):
                NO = 512
                wt = p_wv.tile([P, n_cj, 2, NO], F8, tag="wv")
                nc.sync.dma_start(
                    wt[:],
                    wv_d[:, ts(no, NO)]
                    .rearrange("(c j p) m -> p c j m", j=2, p=P))
                for to in range(n_tk):
                    ps = ps_sh.tile([P, NO], F32, tag="sh")
                    for cj in range(n_cj):
                        nc.tensor.matmul(
                            ps[:], XN8[:, 2 * cj:2 * cj + 2, ts(to, P)],
                            wt[:, cj, :, :],
                            start=(cj == 0), stop=(cj == n_cj - 1),
                            perf_mode=DR)
                    nc.vector.tensor_add(VT[:, to, 8 * no:8 * no + 8, 0:64],
                                         ps[:], bv_bc[:, ts(no, NO)])

            for mo in range(4):
                k_proj(mo)
                q_proj(mo)
            v_proj(0)
            for hp in range(2):
                attn_block(0, hp)
            for mo in range(4, n_dc):
                k_proj(mo)
                q_proj(mo)
            v_proj(1)
            for hp in range(2, 4):
                attn_block(0, hp)

        # ================= Phase B: rest of attention + Wo + MLP ===========
        with tc.tile_pool(name="p_mlp", bufs=1) as p_mlp, \
             tc.tile_pool(name="p_t2", bufs=2) as p_t2, \
             tc.tile_pool(name="p_wos", bufs=1) as p_wos, \
             tc.tile_pool(name="p_w1", bufs=2) as p_w1, \
             tc.tile_pool(name="p_w2", bufs=2) as p_w2, \
             tc.tile_pool(name="p_y1", bufs=1) as p_y1, \
             tc.tile_pool(name="p_out", bufs=2) as p_out:

            XN2 = p_mlp.tile([P, n_dc, Q], F8)
            y1s = {}

            def wo_block(qq):
                qsl = ts(qq, QQ)
                wt_all = p_wos.tile([P, n_cj, 2, n_dc, P], F8, tag="wo")
                nc.sync.dma_start(
                    wt_all[:],
                    wo_d[:, :]
                    .rearrange("(c j p) (mo m) -> p c j mo m", j=2, p=P, m=P))
                for mo in range(n_dc):
                    ps = ps_sh.tile([P, QQ], F32, tag="sh")
                    for cj in range(n_cj):
                        nc.tensor.matmul(
                            ps[:], wt_all[:, cj, :, mo, :],
                            CT8[:, 2 * cj:2 * cj + 2, qsl],
                            start=(cj == 0), stop=(cj == n_cj - 1),
                            perf_mode=DR)
                    nc.vector.affine_then_add(XQ[:, mo, qsl], ps[:],
                                              XQ[:, mo, qsl],
                                              scale=c_wo,
                                              bias=bo_t[:, mo:mo + 1])

            def ln2_block(qq):
                qsl = ts(qq, QQ)
                st2 = ps_sc.tile([P, 2, QQ], F32, tag="ps_s", name="ps_s")
                for dc in range(n_dc):
                    nc.tensor.matmul(st2[:, 0, :], ones_f[:], XQ[:, dc, qsl],
                                     start=(dc == 0), stop=(dc == n_dc - 1))
                    sq = p_t2.tile([P, QQ], BF16, tag="sq2")
                    nc.gpsimd.tensor_mul(sq[:], XQ[:, dc, qsl],
                                         XQ[:, dc, qsl])
                    nc.tensor.matmul(st2[:, 1, :], ones_f[:], sq[:],
                                     start=(dc == 0), stop=(dc == n_dc - 1))
                mbc = p_t2.tile([P, QQ], F32, tag="mbc2")
                nc.vector.tensor_scalar_mul(mbc[:], st2[:, 0, :], inv_d)
                var = p_t2.tile([P, QQ], F32, tag="var2")
                nc.vector.tensor_scalar(var[:], st2[:, 1, :], inv_d, EPS,
                                        op0=ALU.mult, op1=ALU.add)
                m2 = p_t2.tile([P, QQ], F32, tag="tn2")
                nc.vector.tensor_mul(m2[:], mbc[:], mbc[:])
                nc.vector.tensor_sub(var[:], var[:], m2[:])
                # rstd = rsqrt(var) on DVE only: seed from 1/var + Newton
                r = p_t2.tile([P, QQ], F32, tag="rstd2")
                nc.vector.reciprocal_approx_fast(r[:], var[:])
                nc.vector.tensor_scalar(r[:], r[:], 0.72, 0.35,
                                        op0=ALU.mult, op1=ALU.add)
                for _ in range(3):
                    t1 = p_t2.tile([P, QQ], F32, tag="tn2")
                    nc.vector.tensor_mul(t1[:], r[:], r[:])
                    nc.vector.tensor_mul(t1[:], t1[:], var[:])
                    nc.vector.tensor_scalar(t1[:], t1[:], -0.5, 1.5,
                                            op0=ALU.mult, op1=ALU.add)
                    nc.vector.tensor_mul(r[:], r[:], t1[:])
                for dc in range(n_dc):
                    t0 = p_t2.tile([P, QQ], F32, tag="tn2")
                    nc.vector.tensor_sub(t0[:], XQ[:, dc, qsl], mbc[:])
                    nc.gpsimd.tensor_mul(XN2[:, dc, qsl], t0[:], r[:])

            def fc1_block(qq, mo0, mo1):
                qsl = ts(qq, QQ)
                if qq not in y1s:
                    y1s[qq] = p_y1.tile([P, n_mo, QQ], F8, tag="y1",
                                        name="y1")
                Y1 = y1s[qq]
                for mo in range(mo0, mo1):
                    if mo % 8 == 0:
                        wt8 = p_w1.tile([P, n_cj, 2, 8, P], F8, tag="w1")
                        nc.sync.dma_start(
                            wt8[:],
                            w1_d[:, ts(mo // 8, 8 * P)]
                            .rearrange("(c j p) (mo m) -> p c j mo m",
                                       j=2, p=P, m=P))
                    ps = ps_sh.tile([P, QQ], F32, tag="sh")
                    for cj in range(n_cj):
                        nc.tensor.matmul(
                            ps[:], wt8[:, cj, :, mo % 8, :],
                            XN2[:, 2 * cj:2 * cj + 2, qsl],
                            start=(cj == 0), stop=(cj == n_cj - 1),
                            perf_mode=DR)
                    nc.scalar.activation(Y1[:, mo, :], ps[:], AF.Gelu,
                                         bias=b1_t[:, mo:mo + 1],
                                         scale=inv_s1)

            def fc2_block(qq):
                qsl = ts(qq, QQ)
                Y1 = y1s.pop(qq)
                for mo2 in range(n_dc):
                    if mo2 % 2 == 0:
                        wt2 = p_w2.tile([P, n_m2, 2, 2, P], F8, tag="w2")
                        nc.sync.dma_start(
                            wt2[:],
                            w2_d[:, ts(mo2 // 2, 2 * P)]
                            .rearrange("(c j p) (mo m) -> p c j mo m",
                                       j=2, p=P, m=P))
                    ps = ps_sh.tile([P, QQ], F32, tag="sh")
                    for cj in range(n_m2):
                        nc.tensor.matmul(
                            ps[:], wt2[:, cj, :, mo2 % 2, :],
                            Y1[:, 2 * cj:2 * cj + 2, :],
                            start=(cj == 0), stop=(cj == n_m2 - 1),
                            perf_mode=DR)
                    ot = p_out.tile([P, QQ], F32, tag="ot")
                    nc.vector.affine_then_add(ot[:], ps[:], XQ[:, mo2, qsl],
                                              scale=inv_s2,
                                              bias=b2_t[:, mo2:mo2 + 1])
                    nc.sync.dma_start(yT_d[ts(mo2, P), qsl], ot[:])

            for hp in range(4, n_hp):
                attn_block(0, hp)
            wo_block(0)
            attn_block(1, 0)
            attn_block(1, 1)
            ln2_block(0)
            for hp in range(2, n_hp):
                attn_block(1, hp)
            wo_block(1)
            fc1_block(0, 0, n_mo)
            ln2_block(1)
            fc2_block(0)
            fc1_block(1, 0, n_mo)
            fc2_block(1)
    nc.compile()
    return nc


_NC_CACHE = {}


def _get_nc(T, Q, Dm, Hh, Mlp, n_cores,
            scales=(16.0, 16.0, 16.0, 16.0, 16.0, 16.0, 3.5)):
    key = (T, Q, Dm, Hh, Mlp, n_cores, tuple(scales))
    if key not in _NC_CACHE:
        _NC_CACHE[key] = build_bass(T, Q, Dm, Hh, Mlp, n_cores, scales)
    return _NC_CACHE[key]


def _pow2_scale(absmax, target=128.0):
    a = float(absmax)
    if not np.isfinite(a) or a <= 0:
        return 1.0
    return float(2.0 ** math.floor(math.log2(target / a)))


def prepare(inputs):
    """Host-side prep: LN folding, fp8 quantization, per-core input maps."""
    f = lambda k: np.asarray(inputs[k], np.float32)
    x = f("x")
    Bq, Sq, Dq = x.shape
    Qtok = Sq // 2
    g1, b1ln = f("ln1_g"), f("ln1_b")
    g2, b2ln = f("ln2_g"), f("ln2_b")
    Wq, Wk, Wv, Wo = f("Wq"), f("Wk"), f("Wv"), f("Wo")
    W1, W2 = f("W1"), f("W2")
    bq, bk, bv, bo = f("bq"), f("bk"), f("bv"), f("bo")
    b1, b2 = f("b1"), f("b2")

    # fold LN1 gain/bias into QKV, LN2 gain/bias into W1 (exact)
    Wq_e = g1[:, None] * Wq
    Wk_e = g1[:, None] * Wk
    Wv_e = g1[:, None] * Wv
    bq_e = bq + b1ln @ Wq
    bk_e = bk + b1ln @ Wk
    bv_e = bv + b1ln @ Wv
    W1_e = g2[:, None] * W1
    b1_e = b1 + b2ln @ W1

    s_wq = _pow2_scale(np.abs(Wq_e).max())
    s_wk = _pow2_scale(np.abs(Wk_e).max())
    # V result is stored in fp8 still scaled by s_wv: bound both weight and
    # activation range (sigma of v_j ~ col norm of Wv_e, x is LN'd)
    vcol = np.sqrt((Wv_e ** 2).sum(0))
    vmag = max(float((vcol * 8).max()), float(np.abs(bv_e).max() * 4), 1e-6)
    s_wv = min(_pow2_scale(np.abs(Wv_e).max()),
               _pow2_scale(vmag, target=200.0))
    s_wo = _pow2_scale(np.abs(Wo).max())
    s_w1 = _pow2_scale(np.abs(W1_e).max())
    s_w2 = _pow2_scale(np.abs(W2).max())

    # estimate max attention score for the exp shift C (sampled)
    mu = x.mean(-1, keepdims=True)
    va = x.var(-1, keepdims=True)
    xn_h = (x - mu) / np.sqrt(va + EPS)
    qi = xn_h[:, ::89][:, :16].reshape(-1, Dq)
    ki = xn_h[:, ::13][:, :128].reshape(-1, Dq)
    qp = (qi @ Wq_e + bq_e).reshape(Bq, -1, H, Dq // H)
    kp = (ki @ Wk_e + bk_e).reshape(Bq, -1, H, Dq // H)
    sc = np.einsum("bqhd,bkhd->bhqk", qp, kp) / np.sqrt(Dq // H)
    shift_c = float(sc.max() + 2.0 * sc.std() - math.log(32.0))

    scales = (s_wq, s_wk, s_wv, s_wo, s_w1, s_w2, shift_c)
    nc = _get_nc(Sq, Qtok, Dq, H, MLP, N_CORES, scales)

    shared = {
        "wq8": (Wq_e * s_wq).astype(NP_F8),
        "wk8": (Wk_e * s_wk).astype(NP_F8),
        "wv8": (Wv_e * s_wv).astype(NP_F8),
        "wo8": (Wo * s_wo).astype(NP_F8),
        "w18": (W1_e * s_w1).astype(NP_F8),
        "w28": (W2 * s_w2).astype(NP_F8),
        "bq": (bq_e * s_wq).astype(np.float32),
        "bk": (bk_e * s_wk).astype(np.float32),
        "bv16": (bv_e * s_wv).astype(ml_dtypes.bfloat16),
        "bo": bo.astype(np.float32),
        "b1": b1_e.astype(np.float32),
        "b2": b2.astype(np.float32),
        "ones32": np.ones((P, P), np.float32),
    }
    in_maps = []
    for c in range(N_CORES):
        b = c // 2
        half = c % 2
        xb = x[b]
        xr = np.concatenate(
            [xb[half * Qtok:(half + 1) * Qtok],
             xb[(1 - half) * Qtok:(2 - half) * Qtok]], axis=0)
        m = dict(shared)
        m["xT"] = np.ascontiguousarray(xr.T)
        in_maps.append(m)
    return nc, in_maps, Qtok


def unshard(res, Bq, Sq, Dq, Qtok):
    out = np.empty((Bq, Sq, Dq), np.float32)
    for c in range(N_CORES):
        b = c // 2
        half = c % 2
        out[b, half * Qtok:(half + 1) * Qtok, :] = res.results[c]["yT"].T
    return out


def kernel(**inputs):
    x = np.asarray(inputs["x"], np.float32)
    Bq, Sq, Dq = x.shape
    nc, in_maps, Qtok = prepare(inputs)
    res = run_bass_kernel_spmd(nc, in_maps, core_ids=list(range(N_CORES)))
    return unshard(res, Bq, Sq, Dq, Qtok)
